# revision 15
# baseline (speedup 1.0000x reference)
"""Trainium2 Bass kernel for nn_BaseGenerator_38989713113442.

6-layer post-norm transformer encoder (B=8, L=512, E=512, H=8, FFN=2048) with a
gathered per-head attention bias (three 513-entry table lookups over (B,L,L)
index tensors) and an edge-logit head (cumsum scatter + bilinear logits).

Strategy: data-parallel over batch B across the 8 NeuronCores (one row per
core).  Activations are kept transposed on-chip (feature dim on partitions,
L=512 on the free dim) so every matmul consumes the previous output without
transposes.  Matmuls run in fp16 with fp32 PSUM accumulation; layernorm /
softmax bookkeeping stays fp32.  The attention-bias table gathers run on the
GpSimd engine (ap_gather), are summed on DVE, round-trip through a DRAM bounce
buffer to land in (k-partition, q-free) layout, and get causal/padding masks
applied in place.  Host-side work is layout/index preprocessing only (one-hot
encodings of integer inputs, wrapped int16 gather indices, transposed weights,
mask tensors); all FLOPs happen on-device.
"""

import math
import os
import sys
import time

sys.path.insert(0, "/opt/trn_rl_repo")

import numpy as np

import concourse.bacc as bacc
import concourse.mybir as mybir
from concourse.tile import TileContext

B, L = 8, 512
E, H, HD = 512, 8, 64
FF, NL = 2048, 6
V, NRE, RING_START, PADTOK, MAXLEN = 45, 20, 24, 0, 512
NOUT0 = V - NRE  # 25
EBN, FBN = 4, 16  # 128-blocks in E and FF
NEG = float("-inf")
SCALE_E = math.sqrt(float(E))
INV_SQRT_HD = 1.0 / math.sqrt(float(HD))  # folded into k-projection weights
INV_SQRT_E = E ** -0.5

F16 = mybir.dt.float16
F32 = mybir.dt.float32
I16 = mybir.dt.int16
U8 = mybir.dt.uint8
AF = mybir.ActivationFunctionType
ALU = mybir.AluOpType

CH = 4096                      # gather idxs per group per call
NCHUNK = (128 * 256) // CH     # 8 chunks cover 128 k x 256 q per group

_cached = {}


def build_program(debug=False):
    nc = bacc.Bacc("TRN2", target_bir_lowering=False, debug=False)
    dt = nc.dram_tensor

    emb_tab = dt("emb_tab", [128, 5, E], F16, kind="ExternalInput")
    emb_oh = dt("emb_oh", [128, 5, L], F16, kind="ExternalInput")
    wqkv = dt("wqkv", [NL, 128, 4, 1024], F16, kind="ExternalInput")
    bqk = dt("bqk", [128, NL, 8], F32, kind="ExternalInput")
    wv = dt("wv", [NL, 128, 5, E], F16, kind="ExternalInput")
    wout = dt("wout", [NL, 128, 4, E], F16, kind="ExternalInput")
    bout = dt("bout", [128, NL, 4], F32, kind="ExternalInput")
    wf1 = dt("wf1", [NL, 128, 4, FF], F16, kind="ExternalInput")
    bf1 = dt("bf1", [128, NL, 16], F32, kind="ExternalInput")
    wf2 = dt("wf2", [NL, 128, 16, E], F16, kind="ExternalInput")
    bf2 = dt("bf2", [128, NL, 4], F32, kind="ExternalInput")
    lnp = dt("lnp", [128, 13, 2, 4], F32, kind="ExternalInput")
    tabs = dt("tabs", [128, 3, 513], F32, kind="ExternalInput")
    idxw = dt("idxw", [3, NCHUNK, 128, CH // 16], I16, kind="ExternalInput")
    maskck = dt("maskck", [128, 4, L], F16, kind="ExternalInput")
    wgen = dt("wgen", [128, 4, NOUT0], F16, kind="ExternalInput")
    bgen = dt("bgen", [NOUT0, 1], F32, kind="ExternalInput")
    w0r = dt("w0r", [128, 4, E], F16, kind="ExternalInput")
    b0c = dt("b0c", [128, 4], F32, kind="ExternalInput")
    w1r = dt("w1r", [128, 5, E], F16, kind="ExternalInput")
    s16d = dt("s16d", [128, 4, NRE], F16, kind="ExternalInput")
    masku = dt("masku", [V, L], U8, kind="ExternalInput")
    logT = dt("logT", [V, L], F32, kind="ExternalOutput")
    dbg = dt("dbg", [NL + 1, 128, EBN, L], F32, kind="ExternalOutput") if debug else None

    with TileContext(nc) as tc:
        with tc.tile_pool(name="persist", bufs=1) as pp, \
             tc.tile_pool(name="rows", bufs=1) as rowp, \
             tc.tile_pool(name="psmm", bufs=2, space="PSUM") as psmm, \
             tc.tile_pool(name="psaux", bufs=2, space="PSUM") as psaux, \
             tc.tile_pool(name="psrow", bufs=2, space="PSUM") as psrow:

            biasT = pp.tile([128, H, 4, L], F16)        # [k_local, h, kb, q]
            xT32 = pp.tile([128, EBN, L], F32)          # x transposed, fp32
            x16 = pp.tile([128, EBN, L], F16)           # x transposed, fp16
            ones16 = pp.tile([128, L], F16)             # row 0 = 1.0, else 0
            onesc16 = pp.tile([128, 1], F16)            # all 1.0 (stats lhsT)
            onesr16 = rowp.tile([1, 128], F16)          # all 1.0 (bcast lhsT)

            nc.vector.memset(ones16[:], 0.0)
            nc.vector.memset(ones16[0:1, :], 1.0)
            nc.vector.memset(onesc16[:], 1.0)
            nc.vector.memset(onesr16[:], 1.0)

            # ---------------- Phase G: attention bias gather -------------
            with tc.tile_pool(name="gat", bufs=1) as gp, \
                 tc.tile_pool(name="gidx", bufs=3) as gip, \
                 tc.tile_pool(name="gout", bufs=2) as gop, \
                 tc.tile_pool(name="gacc", bufs=2) as gap, \
                 tc.tile_pool(name="gdram", bufs=1, space="DRAM") as gdp:
                tabs_s = gp.tile([128, 3, 513], F32)
                nc.sync.dma_start(tabs_s[:], tabs[:])
                bounce = gdp.tile([NCHUNK, 128, CH], F32)
                for c in range(NCHUNK):
                    gacc = gap.tile([128, CH], F32, tag="gacc")
                    for t in range(3):
                        ix = gip.tile([128, CH // 16], I16, tag="gidx")
                        nc.sync.dma_start(ix[:], idxw[t, c])
                        if t == 0:
                            nc.gpsimd.ap_gather(
                                gacc[:], tabs_s[:, t, :], ix[:],
                                channels=128, num_elems=513, d=1, num_idxs=CH)
                        else:
                            gt = gop.tile([128, CH], F32, tag="gt")
                            nc.gpsimd.ap_gather(
                                gt[:], tabs_s[:, t, :], ix[:],
                                channels=128, num_elems=513, d=1, num_idxs=CH)
                            nc.vector.tensor_tensor(gacc[:], gacc[:], gt[:], op=ALU.add)
                    nc.sync.dma_start(bounce[c], gacc[:])
                # redistribute: bounce[c][16g+h, kk*256+q] -> biasT[16c+kk, h, kb, qh*256+q]
                for c in range(NCHUNK):
                    srcv = bounce[c].rearrange(
                        "(kb qh hh) (kk q) -> kb qh kk hh q", kb=4, qh=2, hh=16, kk=16)
                    dstv = biasT[16 * c:16 * c + 16].rearrange(
                        "p h kb (qh q) -> kb qh p h q", qh=2)
                    for kb in range(4):
                        for qh in range(2):
                            nc.gpsimd.dma_start(
                                dstv[kb, qh], srcv[kb, qh, :, 0:H])
                # combined causal + key-padding additive mask (-inf / 0)
                mck_s = gp.tile([128, 4, L], F16)
                nc.sync.dma_start(mck_s[:], maskck[:])
                for h in range(H):
                    for kb in range(4):
                        nc.vector.tensor_tensor(
                            biasT[:, h, kb, :], biasT[:, h, kb, :],
                            mck_s[:, kb, :], op=ALU.add)

            # ---------------- Phase E: embeddings ------------------------
            with tc.tile_pool(name="emb", bufs=1) as ep:
                et = ep.tile([128, 5, E], F16)
                eo = ep.tile([128, 5, L], F16)
                nc.sync.dma_start(et[:], emb_tab[:])
                nc.sync.dma_start(eo[:], emb_oh[:])
                for eb in range(EBN):
                    ps = psmm.tile([128, L], F32, tag="mm")
                    for kb in range(5):
                        nc.tensor.matmul(
                            ps[:], et[:, kb, eb * 128:(eb + 1) * 128], eo[:, kb, :],
                            start=(kb == 0), stop=(kb == 4))
                    nc.scalar.activation(xT32[:, eb, :], ps[:], AF.Copy, scale=SCALE_E)
                    nc.scalar.activation(x16[:, eb, :], ps[:], AF.Copy, scale=SCALE_E)
            if debug:
                nc.sync.dma_start(dbg[0], xT32[:])

            # ---------------- layers -------------------------------------
            with tc.tile_pool(name="wq", bufs=2) as wqp, \
                 tc.tile_pool(name="wv", bufs=2) as wvp, \
                 tc.tile_pool(name="wo", bufs=2) as wop, \
                 tc.tile_pool(name="wf1", bufs=1) as wf1p, \
                 tc.tile_pool(name="wf2", bufs=1) as wf2p, \
                 tc.tile_pool(name="lay", bufs=1) as lp, \
                 tc.tile_pool(name="att", bufs=2) as ap_, \
                 tc.tile_pool(name="bias_s", bufs=1) as bsp:

                bias_all = bsp.tile([128, 13, 2, 4], F32)
                nc.sync.dma_start(bias_all[:], lnp[:])
                bqk_s = bsp.tile([128, NL, 8], F32)
                nc.sync.dma_start(bqk_s[:], bqk[:])
                bout_s = bsp.tile([128, NL, 4], F32)
                nc.sync.dma_start(bout_s[:], bout[:])
                bf1_s = bsp.tile([128, NL, 16], F32)
                nc.sync.dma_start(bf1_s[:], bf1[:])
                bf2_s = bsp.tile([128, NL, 4], F32)
                nc.sync.dma_start(bf2_s[:], bf2[:])

                def layernorm(src32, ln_idx):
                    """src32 (128, EBN, L) f32 -> writes x16 and xT32 (post-LN)."""
                    sq = lp.tile([128, EBN, L], F16, tag="sq")
                    xp = x16
                    for eb in range(EBN):
                        nc.vector.tensor_tensor(
                            sq[:, eb, :], src32[:, eb, :], src32[:, eb, :], op=ALU.mult)
                        nc.scalar.activation(xp[:, eb, :], src32[:, eb, :], AF.Copy)
                    psm = psrow.tile([1, L], F32, tag="st")
                    for eb in range(EBN):
                        nc.tensor.matmul(psm[:], onesc16[:], xp[:, eb, :],
                                         start=(eb == 0), stop=(eb == 3))
                    psv = psrow.tile([1, L], F32, tag="st")
                    for eb in range(EBN):
                        nc.tensor.matmul(psv[:], onesc16[:], sq[:, eb, :],
                                         start=(eb == 0), stop=(eb == 3))
                    rw = rowp.tile([1, 4, L], F32, tag="lnrows")
                    r16 = rowp.tile([1, 2, L], F16, tag="lnrows16")
                    nc.vector.tensor_scalar_mul(rw[0:1, 0, :], psm[0:1, :], 1.0 / E)
                    nc.vector.tensor_scalar_mul(rw[0:1, 1, :], psv[0:1, :], 1.0 / E)
                    nc.vector.tensor_tensor(rw[0:1, 2, :], rw[0:1, 0, :], rw[0:1, 0, :], op=ALU.mult)
                    nc.vector.tensor_tensor(rw[0:1, 1, :], rw[0:1, 1, :], rw[0:1, 2, :], op=ALU.subtract)
                    nc.vector.tensor_scalar_add(rw[0:1, 1, :], rw[0:1, 1, :], 1e-5)
                    nc.scalar.activation(rw[0:1, 2, :], rw[0:1, 1, :], AF.Sqrt)
                    nc.vector.reciprocal(rw[0:1, 3, :], rw[0:1, 2, :])
                    nc.vector.tensor_copy(r16[0:1, 0, :], rw[0:1, 0, :])   # mean fp16
                    nc.vector.tensor_copy(r16[0:1, 1, :], rw[0:1, 3, :])   # rstd fp16
                    psbm = psaux.tile([128, L], F32, tag="bc")
                    nc.tensor.matmul(psbm[:], onesr16[:], r16[0:1, 0, :], start=True, stop=True)
                    psbr = psaux.tile([128, L], F32, tag="bc")
                    nc.tensor.matmul(psbr[:], onesr16[:], r16[0:1, 1, :], start=True, stop=True)
                    for eb in range(EBN):
                        nc.vector.tensor_tensor(
                            src32[:, eb, :], src32[:, eb, :], psbm[:], op=ALU.subtract)
                        nc.vector.tensor_tensor(
                            src32[:, eb, :], src32[:, eb, :], psbr[:], op=ALU.mult)
                        g_col = bias_all[:, ln_idx, 0, eb:eb + 1]
                        b_col = bias_all[:, ln_idx, 1, eb:eb + 1]
                        nc.scalar.activation(
                            x16[:, eb, :], src32[:, eb, :], AF.Identity, bias=b_col, scale=g_col)
                        nc.scalar.activation(
                            xT32[:, eb, :], src32[:, eb, :], AF.Identity, bias=b_col, scale=g_col)

                for li in range(NL):
                    wq_s = wqp.tile([128, 4, 1024], F16, tag="wq")
                    nc.sync.dma_start(wq_s[:], wqkv[li])
                    wv_s = wvp.tile([128, 5, E], F16, tag="wv")
                    nc.sync.dma_start(wv_s[:], wv[li])
                    wo_s = wop.tile([128, 4, E], F16, tag="wo")
                    nc.sync.dma_start(wo_s[:], wout[li])

                    # q/k projections (k pre-scaled by 1/sqrt(hd) on host)
                    qk = lp.tile([128, 8, L], F16, tag="qk")
                    for m in range(8):
                        ps = psmm.tile([128, L], F32, tag="mm")
                        for kb in range(4):
                            nc.tensor.matmul(
                                ps[:], wq_s[:, kb, m * 128:(m + 1) * 128], x16[:, kb, :],
                                start=(kb == 0), stop=(kb == 3))
                        nc.scalar.activation(qk[:, m, :], ps[:], AF.Identity,
                                             bias=bqk_s[:, li, m:m + 1])
                    # v (untransposed: l on partitions) + ones column for sums
                    v16 = lp.tile([128, 4, H, HD + 1], F16, tag="v16")
                    nc.vector.memset(v16[:, :, :, HD:HD + 1], 1.0)
                    for lb in range(4):
                        ps = psmm.tile([128, L], F32, tag="mm")
                        for kb in range(5):
                            lhs = (x16[:, kb, lb * 128:(lb + 1) * 128] if kb < 4
                                   else ones16[:, lb * 128:(lb + 1) * 128])
                            nc.tensor.matmul(ps[:], lhs, wv_s[:, kb, :],
                                             start=(kb == 0), stop=(kb == 4))
                        nc.scalar.activation(
                            v16[:, lb, :, 0:HD],
                            ps[:].rearrange("p (h d) -> p h d", d=HD), AF.Copy)

                    ctx16 = lp.tile([128, EBN, L], F16, tag="ctx")
                    for h in range(H):
                        po = (h % 2) * 64
                        mq, mk = h // 2, 4 + h // 2
                        aT = ap_.tile([128, 4, L], F16, tag="aT")
                        for kb in range(4):
                            psA = psmm.tile([128, L], F32, tag="mm")
                            nc.tensor.matmul(
                                psA[:],
                                qk[po:po + 64, mk, kb * 128:(kb + 1) * 128],
                                qk[po:po + 64, mq, :],
                                start=True, stop=True)
                            nc.vector.tensor_tensor(
                                psA[:], psA[:], biasT[:, h, kb, :], op=ALU.add)
                            nc.scalar.activation(aT[:, kb, :], psA[:], AF.Exp)
                        psC = psaux.tile([HD + 1, L], F32, tag="ctxp")
                        for kb in range(4):
                            nc.tensor.matmul(psC[:], v16[:, kb, h, :], aT[:, kb, :],
                                             start=(kb == 0), stop=(kb == 3))
                        # reciprocal of sums (row 64) at matching partitions, then
                        # DMA the fp16 row down to partition 0 for the broadcast
                        rc64 = ap_.tile([HD + 1, L], F32, tag="rc64")
                        r1664 = ap_.tile([HD + 1, L], F16, tag="r1664")
                        rrow = ap_.tile([1, L], F16, tag="rrow")
                        nc.vector.reciprocal(rc64[HD:HD + 1, :], psC[HD:HD + 1, :])
                        nc.vector.tensor_copy(r1664[HD:HD + 1, :], rc64[HD:HD + 1, :])
                        nc.sync.dma_start(rrow[:], r1664[HD:HD + 1, :])
                        psR = psaux.tile([128, L], F32, tag="bc")
                        nc.tensor.matmul(psR[:], onesr16[:], rrow[:],
                                         start=True, stop=True)
                        rb16 = ap_.tile([128, L], F16, tag="rb16")
                        nc.scalar.activation(rb16[:], psR[:], AF.Copy)
                        ctxh = ap_.tile([HD, L], F16, tag="ctxh")
                        nc.vector.tensor_tensor(
                            ctxh[:], psC[0:HD, :], rb16[0:HD, :], op=ALU.mult)
                        nc.sync.dma_start(ctx16[po:po + 64, h // 2, :], ctxh[:])

                    res32 = lp.tile([128, EBN, L], F32, tag="res")
                    for eb in range(EBN):
                        ps = psmm.tile([128, L], F32, tag="mm")
                        for kb in range(4):
                            nc.tensor.matmul(
                                ps[:], wo_s[:, kb, eb * 128:(eb + 1) * 128],
                                ctx16[:, kb, :], start=(kb == 0), stop=(kb == 3))
                        nc.scalar.activation(res32[:, eb, :], ps[:], AF.Identity,
                                             bias=bout_s[:, li, eb:eb + 1])
                        nc.vector.tensor_tensor(
                            res32[:, eb, :], res32[:, eb, :], xT32[:, eb, :], op=ALU.add)
                    layernorm(res32, 2 * li)

                    # FFN
                    w1_s = wf1p.tile([128, 4, FF], F16, tag="wf1")
                    nc.sync.dma_start(w1_s[:], wf1[li])
                    hT = lp.tile([128, FBN, L], F16, tag="hT")
                    for fb in range(FBN):
                        ps = psmm.tile([128, L], F32, tag="mm")
                        for kb in range(4):
                            nc.tensor.matmul(
                                ps[:], w1_s[:, kb, fb * 128:(fb + 1) * 128],
                                x16[:, kb, :], start=(kb == 0), stop=(kb == 3))
                        nc.scalar.activation(hT[:, fb, :], ps[:], AF.Gelu,
                                             bias=bf1_s[:, li, fb:fb + 1])
                    w2_s = wf2p.tile([128, 16, E], F16, tag="wf2")
                    nc.sync.dma_start(w2_s[:], wf2[li])
                    for eb in range(EBN):
                        ps = psmm.tile([128, L], F32, tag="mm")
                        for kb in range(16):
                            nc.tensor.matmul(
                                ps[:], w2_s[:, kb, eb * 128:(eb + 1) * 128],
                                hT[:, kb, :], start=(kb == 0), stop=(kb == 15))
                        nc.scalar.activation(res32[:, eb, :], ps[:], AF.Identity,
                                             bias=bf2_s[:, li, eb:eb + 1])
                        nc.vector.tensor_tensor(
                            res32[:, eb, :], res32[:, eb, :], xT32[:, eb, :], op=ALU.add)
                    layernorm(res32, 2 * li + 1)
                    if debug:
                        nc.sync.dma_start(dbg[li + 1], xT32[:])

                # final LN (applied on xT32 itself)
                fin32 = lp.tile([128, EBN, L], F32, tag="res")
                for eb in range(EBN):
                    nc.vector.tensor_copy(fin32[:, eb, :], xT32[:, eb, :])
                layernorm(fin32, 12)

                # ------------- head (reuses layer pool slots) -----------
                if True:
                    logsb0 = lp.tile([NOUT0, L], F32, tag="logsb0")
                    logsb1 = lp.tile([NRE, L], F32, tag="logsb1")
                    wg_s = lp.tile([128, 4, NOUT0], F16, tag="wgen")
                    nc.sync.dma_start(wg_s[:], wgen[:])
                    bg_s = lp.tile([NOUT0, 1], F32, tag="bgen")
                    nc.sync.dma_start(bg_s[:], bgen[:])
                    psG = psaux.tile([NOUT0, L], F32, tag="ctxp")
                    for kb in range(4):
                        nc.tensor.matmul(psG[:], wg_s[:, kb, :], x16[:, kb, :],
                                         start=(kb == 0), stop=(kb == 3))
                    nc.scalar.activation(logsb0[:], psG[:], AF.Identity, bias=bg_s[:])

                    w0_s = wop.tile([128, 4, E], F16, tag="wo")
                    nc.sync.dma_start(w0_s[:], w0r[:])
                    b0_s = lp.tile([128, 4], F32, tag="b0c")
                    nc.sync.dma_start(b0_s[:], b0c[:])
                    o0 = lp.tile([128, EBN, L], F16, tag="ctx")
                    for eb in range(EBN):
                        ps = psmm.tile([128, L], F32, tag="mm")
                        for kb in range(4):
                            nc.tensor.matmul(
                                ps[:], w0_s[:, kb, eb * 128:(eb + 1) * 128],
                                x16[:, kb, :], start=(kb == 0), stop=(kb == 3))
                        nc.scalar.activation(o0[:, eb, :], ps[:], AF.Identity,
                                             bias=b0_s[:, eb:eb + 1])
                    w1_s = wvp.tile([128, 5, E], F16, tag="wv")
                    nc.sync.dma_start(w1_s[:], w1r[:])
                    o1 = lp.tile([128, 4, E], F16, tag="sq")
                    for lb in range(4):
                        ps = psmm.tile([128, L], F32, tag="mm")
                        for kb in range(5):
                            lhs = (x16[:, kb, lb * 128:(lb + 1) * 128] if kb < 4
                                   else ones16[:, lb * 128:(lb + 1) * 128])
                            nc.tensor.matmul(ps[:], lhs, w1_s[:, kb, :],
                                             start=(kb == 0), stop=(kb == 4))
                        nc.vector.tensor_copy(o1[:, lb, :], ps[:])
                    s_s = lp.tile([128, 4, NRE], F16, tag="s16")
                    nc.sync.dma_start(s_s[:], s16d[:])
                    rt = lp.tile([128, EBN, NRE], F16, tag="rt")
                    for eb in range(EBN):
                        ps = psaux.tile([128, NRE], F32, tag="ctxp")
                        for lb in range(4):
                            nc.tensor.matmul(
                                ps[:], o1[:, lb, eb * 128:(eb + 1) * 128],
                                s_s[:, lb, :], start=(lb == 0), stop=(lb == 3))
                        nc.vector.tensor_copy(rt[:, eb, :], ps[:])
                    psL = psaux.tile([NRE, L], F32, tag="ctxp")
                    for eb in range(EBN):
                        nc.tensor.matmul(psL[:], rt[:, eb, :], o0[:, eb, :],
                                         start=(eb == 0), stop=(eb == 3))
                    nc.scalar.activation(logsb1[:], psL[:], AF.Copy,
                                         scale=INV_SQRT_E)
                    # output masking: where(mask, -inf, logits)
                    mk0 = lp.tile([NOUT0, L], U8, tag="mk0")
                    mk1 = lp.tile([NRE, L], U8, tag="mk1")
                    nc.sync.dma_start(mk0[:], masku[0:NOUT0])
                    nc.sync.dma_start(mk1[:], masku[NOUT0:V])
                    ninf = lp.tile([NOUT0, L], F32, tag="ninf")
                    nc.vector.memset(ninf[:], NEG)
                    nc.vector.copy_predicated(logsb0[:], mk0[:], ninf[:])
                    nc.vector.copy_predicated(logsb1[:], mk1[:], ninf[0:NRE, :])
                    nc.sync.dma_start(logT[0:NOUT0, :], logsb0[:])
                    nc.sync.dma_start(logT[NOUT0:V, :], logsb1[:])

    nc.compile()
    return nc


def _to_np(x, dtype=None):
    a = np.asarray(x)
    if dtype is not None:
        a = a.astype(dtype)
    return a


def _blk(a, nb):
    """(nb*128, X) -> (128, nb, X)"""
    return np.ascontiguousarray(a.reshape(nb, 128, -1).transpose(1, 0, 2))


def _prep_shared(ins):
    f = lambda k: _to_np(ins[k], np.float32)
    out = {}
    # embedding table: rows 0..44 tok, 128..639 cnt
    TAB = np.zeros((640, E), np.float32)
    TAB[0:V] = f("tok_emb")
    TAB[128:128 + MAXLEN] = f("cnt_emb")
    out["emb_tab"] = _blk(TAB, 5).astype(np.float16)

    ipw, ipb = f("in_proj_w"), f("in_proj_b")
    wqkv = np.zeros((NL, 128, 4, 1024), np.float16)
    bqk = np.zeros((128, NL, 8), np.float32)
    wv = np.zeros((NL, 128, 5, E), np.float16)
    wout_ = np.zeros((NL, 128, 4, E), np.float16)
    bout = np.zeros((128, NL, 4), np.float32)
    wf1 = np.zeros((NL, 128, 4, FF), np.float16)
    bf1 = np.zeros((128, NL, 16), np.float32)
    wf2 = np.zeros((NL, 128, 16, E), np.float16)
    bf2 = np.zeros((128, NL, 4), np.float32)
    lnp = np.zeros((128, 13, 2, 4), np.float32)
    for i in range(NL):
        wq = ipw[i, 0:E]                     # (512, 512)
        wk = ipw[i, E:2 * E] * INV_SQRT_HD
        cat = np.concatenate([wq, wk], 0).T  # (512, 1024) = lhsT
        wqkv[i] = _blk(cat, 4).astype(np.float16)
        bcat = np.concatenate([ipb[i, 0:E], ipb[i, E:2 * E] * INV_SQRT_HD])
        bqk[:, i, :] = bcat.reshape(8, 128).T
        wvt = np.zeros((640, E), np.float32)
        wvt[0:E] = ipw[i, 2 * E:3 * E].T
        wvt[E] = ipb[i, 2 * E:3 * E]
        wv[i] = _blk(wvt, 5).astype(np.float16)
        wout_[i] = _blk(f("out_w")[i].T, 4).astype(np.float16)
        bout[:, i, :] = f("out_b")[i].reshape(4, 128).T
        wf1[i] = _blk(f("ffn_w1")[i].T, 4).astype(np.float16)
        bf1[:, i, :] = f("ffn_b1")[i].reshape(16, 128).T
        wf2[i] = _blk(f("ffn_w2")[i].T, 16).astype(np.float16)
        bf2[:, i, :] = f("ffn_b2")[i].reshape(4, 128).T
        lnp[:, 2 * i, 0, :] = f("ln1_g")[i].reshape(4, 128).T
        lnp[:, 2 * i, 1, :] = f("ln1_b")[i].reshape(4, 128).T
        lnp[:, 2 * i + 1, 0, :] = f("ln2_g")[i].reshape(4, 128).T
        lnp[:, 2 * i + 1, 1, :] = f("ln2_b")[i].reshape(4, 128).T
    lnp[:, 12, 0, :] = f("fin_g").reshape(4, 128).T
    lnp[:, 12, 1, :] = f("fin_b").reshape(4, 128).T
    out.update(wqkv=wqkv, bqk=bqk, wv=wv, wout=wout_, bout=bout,
               wf1=wf1, bf1=bf1, wf2=wf2, bf2=bf2, lnp=lnp)

    tabs = np.zeros((128, 3, 513), np.float32)
    for t, k in enumerate(["lin_loc_emb", "up_loc_emb", "down_loc_emb"]):
        tb = f(k)  # (513, 8)
        for p in range(128):
            tabs[p, t, :] = tb[:, (p % 16) % 8]
    out["tabs"] = tabs

    out["wgen"] = _blk(f("gen_w").T, 4).astype(np.float16)
    out["bgen"] = f("gen_b").reshape(NOUT0, 1)
    out["w0r"] = _blk(f("ring_w0").T, 4).astype(np.float16)
    out["b0c"] = f("ring_b0").reshape(4, 128).T.copy()
    w1t = np.zeros((640, E), np.float32)
    w1t[0:E] = f("ring_w1").T
    w1t[E] = f("ring_b1")
    out["w1r"] = _blk(w1t, 5).astype(np.float16)
    return out


def _prep_core(ins, b):
    seq = _to_np(ins["sequences"], np.int64)[b]
    cnt = _to_np(ins["count_sequences"], np.int64)[b]
    gm = _to_np(ins["graph_mask_sequences"], bool)[b]
    vm = _to_np(ins["valence_mask_sequences"], bool)[b]
    out = {}
    OH = np.zeros((640, L), np.float32)
    OH[seq, np.arange(L)] = 1.0
    OH[128 + cnt, np.arange(L)] = 1.0
    out["emb_oh"] = _blk(OH, 5).astype(np.float16)

    idxw = np.zeros((3, NCHUNK, 128, CH // 16), np.int16)
    for t, k in enumerate(["linear_loc_squares", "up_loc_squares", "down_loc_squares"]):
        IDX = _to_np(ins[k], np.int64)[b]  # (q, k)
        for g in range(8):
            kb, qh = g // 2, g % 2
            sub = IDX[256 * qh:256 * qh + 256, 128 * kb:128 * kb + 128]
            flat = sub.T.reshape(-1)  # j = k_local*256 + q_local
            for c in range(NCHUNK):
                sl = flat[c * CH:(c + 1) * CH]
                idxw[t, c, 16 * g:16 * g + 16, :] = sl.reshape(CH // 16, 16).T
    out["idxw"] = idxw

    # combined causal + key-padding additive mask, (k_local, kb, q) layout
    kg = np.arange(L)
    masked = (kg[:, None] > kg[None, :]) | (seq == PADTOK)[:, None]  # (k, q)
    mck = np.where(masked, NEG, 0.0).astype(np.float16)
    out["maskck"] = np.ascontiguousarray(mck.reshape(4, 128, L).transpose(1, 0, 2))

    ring = (seq == RING_START)
    cs = np.cumsum(ring.astype(np.int64))
    S = np.zeros((L, NRE), np.float32)
    for l in range(L):
        if ring[l] and cs[l] <= NRE:
            S[l, cs[l] - 1] = 1.0
    out["s16d"] = _blk(S, 4).astype(np.float16)

    out["masku"] = np.ascontiguousarray((gm | vm).T).astype(np.uint8)
    return out


last_exec_s = None


def kernel(**inputs):
    global last_exec_s
    if "nc" not in _cached:
        _cached["nc"] = build_program(debug=False)
    nc = _cached["nc"]

    shared = _prep_shared(inputs)
    in_maps = []
    for b in range(B):
        m = dict(shared)
        m.update(_prep_core(inputs, b))
        in_maps.append(m)

    if os.environ.get("KERNEL_SIM"):
        from concourse.bass_interp import CoreSim
        outs = []
        for b in range(B):
            sim = CoreSim(nc, require_finite=False, require_nnan=False)
            for k, v in in_maps[b].items():
                sim.tensor(k)[:] = v
            sim.simulate()
            outs.append(np.array(sim.tensor("logT")).T)
        return np.stack(outs).astype(np.float32)

    from concourse.bass_utils import run_bass_kernel_spmd
    t0 = time.time()
    res = run_bass_kernel_spmd(nc, in_maps, list(range(B)))
    last_exec_s = time.time() - t0
    return np.stack([res.results[b]["logT"].T for b in range(B)]).astype(np.float32)


# revision 17
# speedup vs baseline: 66.0299x; 66.0299x over previous
"""Trainium2 Bass kernel for nn_BaseGenerator_38989713113442.

6-layer post-norm transformer encoder (B=8, L=512, E=512, H=8, FFN=2048) with a
gathered per-head attention bias (three 513-entry table lookups over (B,L,L)
index tensors) and an edge-logit head (cumsum scatter + bilinear logits).

Strategy: data-parallel over batch B across the 8 NeuronCores (one row per
core).  Activations are kept transposed on-chip (feature dim on partitions,
L=512 on the free dim) so every matmul consumes the previous output without
transposes.  Matmuls run in fp16 with fp32 PSUM accumulation; layernorm /
softmax bookkeeping stays fp32.  The attention-bias table gathers run on the
GpSimd engine (ap_gather), are summed on DVE, round-trip through a DRAM bounce
buffer to land in (k-partition, q-free) layout, and get causal/padding masks
applied in place.  Host-side work is layout/index preprocessing only (one-hot
encodings of integer inputs, wrapped int16 gather indices, transposed weights,
mask tensors); all FLOPs happen on-device.
"""

import math
import os
import sys
import time

sys.path.insert(0, "/opt/trn_rl_repo")

import numpy as np

import concourse.bacc as bacc
import concourse.mybir as mybir
from concourse.tile import TileContext

B, L = 8, 512
E, H, HD = 512, 8, 64
FF, NL = 2048, 6
V, NRE, RING_START, PADTOK, MAXLEN = 45, 20, 24, 0, 512
NOUT0 = V - NRE  # 25
EBN, FBN = 4, 16  # 128-blocks in E and FF
NEG = float("-inf")
SCALE_E = math.sqrt(float(E))
INV_SQRT_HD = 1.0 / math.sqrt(float(HD))  # folded into k-projection weights
INV_SQRT_E = E ** -0.5

F16 = mybir.dt.float16
F32 = mybir.dt.float32
I16 = mybir.dt.int16
U8 = mybir.dt.uint8
AF = mybir.ActivationFunctionType
ALU = mybir.AluOpType

CH = 4096                      # gather idxs per group per call
NCHUNK = (128 * 256) // CH     # 8 chunks cover 128 k x 256 q per group

_cached = {}


def build_program(debug=False):
    nc = bacc.Bacc("TRN2", target_bir_lowering=False, debug=False)
    dt = nc.dram_tensor

    emb_tab = dt("emb_tab", [128, 5, E], F16, kind="ExternalInput")
    emb_oh = dt("emb_oh", [128, 5, L], F16, kind="ExternalInput")
    wqkv = dt("wqkv", [NL, 128, 4, 1024], F16, kind="ExternalInput")
    bqk = dt("bqk", [128, NL, 8], F32, kind="ExternalInput")
    wv = dt("wv", [NL, 128, 5, E], F16, kind="ExternalInput")
    wout = dt("wout", [NL, 128, 4, E], F16, kind="ExternalInput")
    bout = dt("bout", [128, NL, 4], F32, kind="ExternalInput")
    wf1 = dt("wf1", [NL, 128, 4, FF], F16, kind="ExternalInput")
    bf1 = dt("bf1", [128, NL, 16], F32, kind="ExternalInput")
    wf2 = dt("wf2", [NL, 128, 16, E], F16, kind="ExternalInput")
    bf2 = dt("bf2", [128, NL, 4], F32, kind="ExternalInput")
    lnp = dt("lnp", [128, 13, 2, 4], F32, kind="ExternalInput")
    tabs = dt("tabs", [128, 3, 513], F32, kind="ExternalInput")
    idxw = dt("idxw", [3, NCHUNK, 128, CH // 16], I16, kind="ExternalInput")
    maskck = dt("maskck", [128, 4, L], F16, kind="ExternalInput")
    wgen = dt("wgen", [128, 4, NOUT0], F16, kind="ExternalInput")
    bgen = dt("bgen", [NOUT0, 1], F32, kind="ExternalInput")
    w0r = dt("w0r", [128, 4, E], F16, kind="ExternalInput")
    b0c = dt("b0c", [128, 4], F32, kind="ExternalInput")
    w1r = dt("w1r", [128, 5, E], F16, kind="ExternalInput")
    s16d = dt("s16d", [128, 4, NRE], F16, kind="ExternalInput")
    masku = dt("masku", [V, L], U8, kind="ExternalInput")
    logT = dt("logT", [V, L], F32, kind="ExternalOutput")
    dbg = dt("dbg", [NL + 1, 128, EBN, L], F32, kind="ExternalOutput") if debug else None

    with TileContext(nc) as tc:
        with tc.tile_pool(name="persist", bufs=1) as pp, \
             tc.tile_pool(name="rows", bufs=1) as rowp, \
             tc.tile_pool(name="psmm", bufs=2, space="PSUM") as psmm, \
             tc.tile_pool(name="psaux", bufs=2, space="PSUM") as psaux, \
             tc.tile_pool(name="psrow", bufs=2, space="PSUM") as psrow:

            biasT = pp.tile([128, H, 4, L], F16)        # [k_local, h, kb, q]
            xT32 = pp.tile([128, EBN, L], F32)          # x transposed, fp32
            x16 = pp.tile([128, EBN, L], F16)           # x transposed, fp16
            ones16 = pp.tile([128, L], F16)             # row 0 = 1.0, else 0
            onesc16 = pp.tile([128, 1], F16)            # all 1.0 (stats lhsT)
            onesr16 = rowp.tile([1, 128], F16)          # all 1.0 (bcast lhsT)

            nc.vector.memset(ones16[:], 0.0)
            nc.vector.memset(ones16[0:1, :], 1.0)
            nc.vector.memset(onesc16[:], 1.0)
            nc.vector.memset(onesr16[:], 1.0)

            # ---------------- Phase G: attention bias gather -------------
            with tc.tile_pool(name="gat", bufs=1) as gp, \
                 tc.tile_pool(name="gidx", bufs=3) as gip, \
                 tc.tile_pool(name="gout", bufs=2) as gop, \
                 tc.tile_pool(name="gacc", bufs=2) as gap, \
                 tc.tile_pool(name="gdram", bufs=1, space="DRAM") as gdp:
                tabs_s = gp.tile([128, 3, 513], F32)
                nc.sync.dma_start(tabs_s[:], tabs[:])
                bounce = gdp.tile([NCHUNK, 128, CH], F32)
                for c in range(NCHUNK):
                    gacc = gap.tile([128, CH], F32, tag="gacc")
                    for t in range(3):
                        ix = gip.tile([128, CH // 16], I16, tag="gidx")
                        nc.sync.dma_start(ix[:], idxw[t, c])
                        if t == 0:
                            nc.gpsimd.ap_gather(
                                gacc[:], tabs_s[:, t, :], ix[:],
                                channels=128, num_elems=513, d=1, num_idxs=CH)
                        else:
                            gt = gop.tile([128, CH], F32, tag="gt")
                            nc.gpsimd.ap_gather(
                                gt[:], tabs_s[:, t, :], ix[:],
                                channels=128, num_elems=513, d=1, num_idxs=CH)
                            nc.vector.tensor_tensor(gacc[:], gacc[:], gt[:], op=ALU.add)
                    nc.sync.dma_start(bounce[c], gacc[:])
                # redistribute: bounce[c][16g+h, kk*256+q] -> biasT[16c+kk, h, kb, qh*256+q]
                for c in range(NCHUNK):
                    srcv = bounce[c].rearrange(
                        "(kb qh hh) (kk q) -> kb qh kk hh q", kb=4, qh=2, hh=16, kk=16)
                    dstv = biasT[16 * c:16 * c + 16].rearrange(
                        "p h kb (qh q) -> kb qh p h q", qh=2)
                    for kb in range(4):
                        for qh in range(2):
                            nc.gpsimd.dma_start(
                                dstv[kb, qh], srcv[kb, qh, :, 0:H])
                # combined causal + key-padding additive mask (-inf / 0)
                mck_s = gp.tile([128, 4, L], F16)
                nc.sync.dma_start(mck_s[:], maskck[:])
                for h in range(H):
                    for kb in range(4):
                        nc.vector.tensor_tensor(
                            biasT[:, h, kb, :], biasT[:, h, kb, :],
                            mck_s[:, kb, :], op=ALU.add)

            # ---------------- Phase E: embeddings ------------------------
            with tc.tile_pool(name="emb", bufs=1) as ep:
                et = ep.tile([128, 5, E], F16)
                eo = ep.tile([128, 5, L], F16)
                nc.sync.dma_start(et[:], emb_tab[:])
                nc.sync.dma_start(eo[:], emb_oh[:])
                for eb in range(EBN):
                    ps = psmm.tile([128, L], F32, tag="mm")
                    for kb in range(5):
                        nc.tensor.matmul(
                            ps[:], et[:, kb, eb * 128:(eb + 1) * 128], eo[:, kb, :],
                            start=(kb == 0), stop=(kb == 4))
                    nc.scalar.activation(xT32[:, eb, :], ps[:], AF.Copy, scale=SCALE_E)
                    nc.scalar.activation(x16[:, eb, :], ps[:], AF.Copy, scale=SCALE_E)
            if debug:
                nc.sync.dma_start(dbg[0], xT32[:])

            # ---------------- layers -------------------------------------
            with tc.tile_pool(name="wq", bufs=2) as wqp, \
                 tc.tile_pool(name="wv", bufs=2) as wvp, \
                 tc.tile_pool(name="wo", bufs=2) as wop, \
                 tc.tile_pool(name="wf1", bufs=1) as wf1p, \
                 tc.tile_pool(name="wf2", bufs=1) as wf2p, \
                 tc.tile_pool(name="lay", bufs=1) as lp, \
                 tc.tile_pool(name="att", bufs=2) as ap_, \
                 tc.tile_pool(name="bias_s", bufs=1) as bsp:

                bias_all = bsp.tile([128, 13, 2, 4], F32)
                nc.sync.dma_start(bias_all[:], lnp[:])
                bqk_s = bsp.tile([128, NL, 8], F32)
                nc.sync.dma_start(bqk_s[:], bqk[:])
                bout_s = bsp.tile([128, NL, 4], F32)
                nc.sync.dma_start(bout_s[:], bout[:])
                bf1_s = bsp.tile([128, NL, 16], F32)
                nc.sync.dma_start(bf1_s[:], bf1[:])
                bf2_s = bsp.tile([128, NL, 4], F32)
                nc.sync.dma_start(bf2_s[:], bf2[:])

                def layernorm(src32, ln_idx):
                    """src32 (128, EBN, L) f32 -> writes x16 and xT32 (post-LN)."""
                    sq = lp.tile([128, EBN, L], F16, tag="sq")
                    xp = x16
                    for eb in range(EBN):
                        nc.vector.tensor_tensor(
                            sq[:, eb, :], src32[:, eb, :], src32[:, eb, :], op=ALU.mult)
                        nc.scalar.activation(xp[:, eb, :], src32[:, eb, :], AF.Copy)
                    psm = psrow.tile([1, L], F32, tag="st")
                    for eb in range(EBN):
                        nc.tensor.matmul(psm[:], onesc16[:], xp[:, eb, :],
                                         start=(eb == 0), stop=(eb == 3))
                    psv = psrow.tile([1, L], F32, tag="st")
                    for eb in range(EBN):
                        nc.tensor.matmul(psv[:], onesc16[:], sq[:, eb, :],
                                         start=(eb == 0), stop=(eb == 3))
                    rw = rowp.tile([1, 4, L], F32, tag="lnrows")
                    r16 = rowp.tile([1, 2, L], F16, tag="lnrows16")
                    nc.vector.tensor_scalar_mul(rw[0:1, 0, :], psm[0:1, :], 1.0 / E)
                    nc.vector.tensor_scalar_mul(rw[0:1, 1, :], psv[0:1, :], 1.0 / E)
                    nc.vector.tensor_tensor(rw[0:1, 2, :], rw[0:1, 0, :], rw[0:1, 0, :], op=ALU.mult)
                    nc.vector.tensor_tensor(rw[0:1, 1, :], rw[0:1, 1, :], rw[0:1, 2, :], op=ALU.subtract)
                    nc.vector.tensor_scalar_add(rw[0:1, 1, :], rw[0:1, 1, :], 1e-5)
                    nc.scalar.activation(rw[0:1, 2, :], rw[0:1, 1, :], AF.Sqrt)
                    nc.vector.reciprocal(rw[0:1, 3, :], rw[0:1, 2, :])
                    nc.vector.tensor_copy(r16[0:1, 0, :], rw[0:1, 0, :])   # mean fp16
                    nc.vector.tensor_copy(r16[0:1, 1, :], rw[0:1, 3, :])   # rstd fp16
                    psbm = psaux.tile([128, L], F32, tag="bc")
                    nc.tensor.matmul(psbm[:], onesr16[:], r16[0:1, 0, :], start=True, stop=True)
                    psbr = psaux.tile([128, L], F32, tag="bc")
                    nc.tensor.matmul(psbr[:], onesr16[:], r16[0:1, 1, :], start=True, stop=True)
                    for eb in range(EBN):
                        nc.vector.tensor_tensor(
                            src32[:, eb, :], src32[:, eb, :], psbm[:], op=ALU.subtract)
                        nc.vector.tensor_tensor(
                            src32[:, eb, :], src32[:, eb, :], psbr[:], op=ALU.mult)
                        g_col = bias_all[:, ln_idx, 0, eb:eb + 1]
                        b_col = bias_all[:, ln_idx, 1, eb:eb + 1]
                        nc.scalar.activation(
                            x16[:, eb, :], src32[:, eb, :], AF.Identity, bias=b_col, scale=g_col)
                        nc.scalar.activation(
                            xT32[:, eb, :], src32[:, eb, :], AF.Identity, bias=b_col, scale=g_col)

                for li in range(NL):
                    wq_s = wqp.tile([128, 4, 1024], F16, tag="wq")
                    nc.sync.dma_start(wq_s[:], wqkv[li])
                    wv_s = wvp.tile([128, 5, E], F16, tag="wv")
                    nc.sync.dma_start(wv_s[:], wv[li])
                    wo_s = wop.tile([128, 4, E], F16, tag="wo")
                    nc.sync.dma_start(wo_s[:], wout[li])

                    # q/k projections (k pre-scaled by 1/sqrt(hd) on host)
                    qk = lp.tile([128, 8, L], F16, tag="qk")
                    for m in range(8):
                        ps = psmm.tile([128, L], F32, tag="mm")
                        for kb in range(4):
                            nc.tensor.matmul(
                                ps[:], wq_s[:, kb, m * 128:(m + 1) * 128], x16[:, kb, :],
                                start=(kb == 0), stop=(kb == 3))
                        nc.scalar.activation(qk[:, m, :], ps[:], AF.Identity,
                                             bias=bqk_s[:, li, m:m + 1])
                    # v (untransposed: l on partitions) + ones column for sums
                    v16 = lp.tile([128, 4, H, HD + 1], F16, tag="v16")
                    nc.vector.memset(v16[:, :, :, HD:HD + 1], 1.0)
                    for lb in range(4):
                        ps = psmm.tile([128, L], F32, tag="mm")
                        for kb in range(5):
                            lhs = (x16[:, kb, lb * 128:(lb + 1) * 128] if kb < 4
                                   else ones16[:, lb * 128:(lb + 1) * 128])
                            nc.tensor.matmul(ps[:], lhs, wv_s[:, kb, :],
                                             start=(kb == 0), stop=(kb == 4))
                        nc.scalar.activation(
                            v16[:, lb, :, 0:HD],
                            ps[:].rearrange("p (h d) -> p h d", d=HD), AF.Copy)

                    ctx16 = lp.tile([128, EBN, L], F16, tag="ctx")
                    for h in range(H):
                        po = (h % 2) * 64
                        mq, mk = h // 2, 4 + h // 2
                        aT = ap_.tile([128, 4, L], F16, tag="aT")
                        for kb in range(4):
                            psA = psmm.tile([128, L], F32, tag="mm")
                            nc.tensor.matmul(
                                psA[:],
                                qk[po:po + 64, mk, kb * 128:(kb + 1) * 128],
                                qk[po:po + 64, mq, :],
                                start=True, stop=True)
                            nc.vector.tensor_tensor(
                                psA[:], psA[:], biasT[:, h, kb, :], op=ALU.add)
                            nc.scalar.activation(aT[:, kb, :], psA[:], AF.Exp)
                        psC = psaux.tile([HD + 1, L], F32, tag="ctxp")
                        for kb in range(4):
                            nc.tensor.matmul(psC[:], v16[:, kb, h, :], aT[:, kb, :],
                                             start=(kb == 0), stop=(kb == 3))
                        # reciprocal of sums (row 64) at matching partitions, then
                        # DMA the fp16 row down to partition 0 for the broadcast
                        rc64 = ap_.tile([HD + 1, L], F32, tag="rc64")
                        r1664 = ap_.tile([HD + 1, L], F16, tag="r1664")
                        rrow = ap_.tile([1, L], F16, tag="rrow")
                        nc.vector.reciprocal(rc64[HD:HD + 1, :], psC[HD:HD + 1, :])
                        nc.vector.tensor_copy(r1664[HD:HD + 1, :], rc64[HD:HD + 1, :])
                        nc.sync.dma_start(rrow[:], r1664[HD:HD + 1, :])
                        psR = psaux.tile([128, L], F32, tag="bc")
                        nc.tensor.matmul(psR[:], onesr16[:], rrow[:],
                                         start=True, stop=True)
                        rb16 = ap_.tile([128, L], F16, tag="rb16")
                        nc.scalar.activation(rb16[:], psR[:], AF.Copy)
                        ctxh = ap_.tile([HD, L], F16, tag="ctxh")
                        nc.vector.tensor_tensor(
                            ctxh[:], psC[0:HD, :], rb16[0:HD, :], op=ALU.mult)
                        nc.sync.dma_start(ctx16[po:po + 64, h // 2, :], ctxh[:])

                    res32 = lp.tile([128, EBN, L], F32, tag="res")
                    for eb in range(EBN):
                        ps = psmm.tile([128, L], F32, tag="mm")
                        for kb in range(4):
                            nc.tensor.matmul(
                                ps[:], wo_s[:, kb, eb * 128:(eb + 1) * 128],
                                ctx16[:, kb, :], start=(kb == 0), stop=(kb == 3))
                        nc.scalar.activation(res32[:, eb, :], ps[:], AF.Identity,
                                             bias=bout_s[:, li, eb:eb + 1])
                        nc.vector.tensor_tensor(
                            res32[:, eb, :], res32[:, eb, :], xT32[:, eb, :], op=ALU.add)
                    layernorm(res32, 2 * li)

                    # FFN
                    w1_s = wf1p.tile([128, 4, FF], F16, tag="wf1")
                    nc.sync.dma_start(w1_s[:], wf1[li])
                    hT = lp.tile([128, FBN, L], F16, tag="hT")
                    for fb in range(FBN):
                        ps = psmm.tile([128, L], F32, tag="mm")
                        for kb in range(4):
                            nc.tensor.matmul(
                                ps[:], w1_s[:, kb, fb * 128:(fb + 1) * 128],
                                x16[:, kb, :], start=(kb == 0), stop=(kb == 3))
                        nc.scalar.activation(hT[:, fb, :], ps[:], AF.Gelu,
                                             bias=bf1_s[:, li, fb:fb + 1])
                    w2_s = wf2p.tile([128, 16, E], F16, tag="wf2")
                    nc.sync.dma_start(w2_s[:], wf2[li])
                    for eb in range(EBN):
                        ps = psmm.tile([128, L], F32, tag="mm")
                        for kb in range(16):
                            nc.tensor.matmul(
                                ps[:], w2_s[:, kb, eb * 128:(eb + 1) * 128],
                                hT[:, kb, :], start=(kb == 0), stop=(kb == 15))
                        nc.scalar.activation(res32[:, eb, :], ps[:], AF.Identity,
                                             bias=bf2_s[:, li, eb:eb + 1])
                        nc.vector.tensor_tensor(
                            res32[:, eb, :], res32[:, eb, :], xT32[:, eb, :], op=ALU.add)
                    layernorm(res32, 2 * li + 1)
                    if debug:
                        nc.sync.dma_start(dbg[li + 1], xT32[:])

                # final LN (applied on xT32 itself)
                fin32 = lp.tile([128, EBN, L], F32, tag="res")
                for eb in range(EBN):
                    nc.vector.tensor_copy(fin32[:, eb, :], xT32[:, eb, :])
                layernorm(fin32, 12)

                # ------------- head (reuses layer pool slots) -----------
                if True:
                    logsb0 = lp.tile([NOUT0, L], F32, tag="logsb0")
                    logsb1 = lp.tile([NRE, L], F32, tag="logsb1")
                    wg_s = lp.tile([128, 4, NOUT0], F16, tag="wgen")
                    nc.sync.dma_start(wg_s[:], wgen[:])
                    bg_s = lp.tile([NOUT0, 1], F32, tag="bgen")
                    nc.sync.dma_start(bg_s[:], bgen[:])
                    psG = psaux.tile([NOUT0, L], F32, tag="ctxp")
                    for kb in range(4):
                        nc.tensor.matmul(psG[:], wg_s[:, kb, :], x16[:, kb, :],
                                         start=(kb == 0), stop=(kb == 3))
                    nc.scalar.activation(logsb0[:], psG[:], AF.Identity, bias=bg_s[:])

                    w0_s = wop.tile([128, 4, E], F16, tag="wo")
                    nc.sync.dma_start(w0_s[:], w0r[:])
                    b0_s = lp.tile([128, 4], F32, tag="b0c")
                    nc.sync.dma_start(b0_s[:], b0c[:])
                    o0 = lp.tile([128, EBN, L], F16, tag="ctx")
                    for eb in range(EBN):
                        ps = psmm.tile([128, L], F32, tag="mm")
                        for kb in range(4):
                            nc.tensor.matmul(
                                ps[:], w0_s[:, kb, eb * 128:(eb + 1) * 128],
                                x16[:, kb, :], start=(kb == 0), stop=(kb == 3))
                        nc.scalar.activation(o0[:, eb, :], ps[:], AF.Identity,
                                             bias=b0_s[:, eb:eb + 1])
                    w1_s = wvp.tile([128, 5, E], F16, tag="wv")
                    nc.sync.dma_start(w1_s[:], w1r[:])
                    o1 = lp.tile([128, 4, E], F16, tag="sq")
                    for lb in range(4):
                        ps = psmm.tile([128, L], F32, tag="mm")
                        for kb in range(5):
                            lhs = (x16[:, kb, lb * 128:(lb + 1) * 128] if kb < 4
                                   else ones16[:, lb * 128:(lb + 1) * 128])
                            nc.tensor.matmul(ps[:], lhs, w1_s[:, kb, :],
                                             start=(kb == 0), stop=(kb == 4))
                        nc.vector.tensor_copy(o1[:, lb, :], ps[:])
                    s_s = lp.tile([128, 4, NRE], F16, tag="s16")
                    nc.sync.dma_start(s_s[:], s16d[:])
                    rt = lp.tile([128, EBN, NRE], F16, tag="rt")
                    for eb in range(EBN):
                        ps = psaux.tile([128, NRE], F32, tag="ctxp")
                        for lb in range(4):
                            nc.tensor.matmul(
                                ps[:], o1[:, lb, eb * 128:(eb + 1) * 128],
                                s_s[:, lb, :], start=(lb == 0), stop=(lb == 3))
                        nc.vector.tensor_copy(rt[:, eb, :], ps[:])
                    psL = psaux.tile([NRE, L], F32, tag="ctxp")
                    for eb in range(EBN):
                        nc.tensor.matmul(psL[:], rt[:, eb, :], o0[:, eb, :],
                                         start=(eb == 0), stop=(eb == 3))
                    nc.scalar.activation(logsb1[:], psL[:], AF.Copy,
                                         scale=INV_SQRT_E)
                    # output masking: where(mask, -inf, logits)
                    mk0 = lp.tile([NOUT0, L], U8, tag="mk0")
                    mk1 = lp.tile([NRE, L], U8, tag="mk1")
                    nc.sync.dma_start(mk0[:], masku[0:NOUT0])
                    nc.sync.dma_start(mk1[:], masku[NOUT0:V])
                    ninf = lp.tile([NOUT0, L], F32, tag="ninf")
                    nc.vector.memset(ninf[:], NEG)
                    nc.vector.copy_predicated(logsb0[:], mk0[:], ninf[:])
                    nc.vector.copy_predicated(logsb1[:], mk1[:], ninf[0:NRE, :])
                    nc.sync.dma_start(logT[0:NOUT0, :], logsb0[:])
                    nc.sync.dma_start(logT[NOUT0:V, :], logsb1[:])

    nc.compile()
    return nc


def _to_np(x, dtype=None):
    a = np.asarray(x)
    if dtype is not None:
        a = a.astype(dtype)
    return a


def _blk(a, nb):
    """(nb*128, X) -> (128, nb, X)"""
    return np.ascontiguousarray(a.reshape(nb, 128, -1).transpose(1, 0, 2))


def _prep_shared(ins):
    f = lambda k: _to_np(ins[k], np.float32)
    out = {}
    # embedding table: rows 0..44 tok, 128..639 cnt
    TAB = np.zeros((640, E), np.float32)
    TAB[0:V] = f("tok_emb")
    TAB[128:128 + MAXLEN] = f("cnt_emb")
    out["emb_tab"] = _blk(TAB, 5).astype(np.float16)

    ipw, ipb = f("in_proj_w"), f("in_proj_b")
    wqkv = np.zeros((NL, 128, 4, 1024), np.float16)
    bqk = np.zeros((128, NL, 8), np.float32)
    wv = np.zeros((NL, 128, 5, E), np.float16)
    wout_ = np.zeros((NL, 128, 4, E), np.float16)
    bout = np.zeros((128, NL, 4), np.float32)
    wf1 = np.zeros((NL, 128, 4, FF), np.float16)
    bf1 = np.zeros((128, NL, 16), np.float32)
    wf2 = np.zeros((NL, 128, 16, E), np.float16)
    bf2 = np.zeros((128, NL, 4), np.float32)
    lnp = np.zeros((128, 13, 2, 4), np.float32)
    for i in range(NL):
        wq = ipw[i, 0:E]                     # (512, 512)
        wk = ipw[i, E:2 * E] * INV_SQRT_HD
        cat = np.concatenate([wq, wk], 0).T  # (512, 1024) = lhsT
        wqkv[i] = _blk(cat, 4).astype(np.float16)
        bcat = np.concatenate([ipb[i, 0:E], ipb[i, E:2 * E] * INV_SQRT_HD])
        bqk[:, i, :] = bcat.reshape(8, 128).T
        wvt = np.zeros((640, E), np.float32)
        wvt[0:E] = ipw[i, 2 * E:3 * E].T
        wvt[E] = ipb[i, 2 * E:3 * E]
        wv[i] = _blk(wvt, 5).astype(np.float16)
        wout_[i] = _blk(f("out_w")[i].T, 4).astype(np.float16)
        bout[:, i, :] = f("out_b")[i].reshape(4, 128).T
        wf1[i] = _blk(f("ffn_w1")[i].T, 4).astype(np.float16)
        bf1[:, i, :] = f("ffn_b1")[i].reshape(16, 128).T
        wf2[i] = _blk(f("ffn_w2")[i].T, 16).astype(np.float16)
        bf2[:, i, :] = f("ffn_b2")[i].reshape(4, 128).T
        lnp[:, 2 * i, 0, :] = f("ln1_g")[i].reshape(4, 128).T
        lnp[:, 2 * i, 1, :] = f("ln1_b")[i].reshape(4, 128).T
        lnp[:, 2 * i + 1, 0, :] = f("ln2_g")[i].reshape(4, 128).T
        lnp[:, 2 * i + 1, 1, :] = f("ln2_b")[i].reshape(4, 128).T
    lnp[:, 12, 0, :] = f("fin_g").reshape(4, 128).T
    lnp[:, 12, 1, :] = f("fin_b").reshape(4, 128).T
    out.update(wqkv=wqkv, bqk=bqk, wv=wv, wout=wout_, bout=bout,
               wf1=wf1, bf1=bf1, wf2=wf2, bf2=bf2, lnp=lnp)

    tabs = np.zeros((128, 3, 513), np.float32)
    for t, k in enumerate(["lin_loc_emb", "up_loc_emb", "down_loc_emb"]):
        tb = f(k)  # (513, 8)
        for p in range(128):
            tabs[p, t, :] = tb[:, (p % 16) % 8]
    out["tabs"] = tabs

    out["wgen"] = _blk(f("gen_w").T, 4).astype(np.float16)
    out["bgen"] = f("gen_b").reshape(NOUT0, 1)
    out["w0r"] = _blk(f("ring_w0").T, 4).astype(np.float16)
    out["b0c"] = f("ring_b0").reshape(4, 128).T.copy()
    w1t = np.zeros((640, E), np.float32)
    w1t[0:E] = f("ring_w1").T
    w1t[E] = f("ring_b1")
    out["w1r"] = _blk(w1t, 5).astype(np.float16)
    return out


def _prep_core(ins, b):
    seq = _to_np(ins["sequences"], np.int64)[b]
    cnt = _to_np(ins["count_sequences"], np.int64)[b]
    gm = _to_np(ins["graph_mask_sequences"], bool)[b]
    vm = _to_np(ins["valence_mask_sequences"], bool)[b]
    out = {}
    OH = np.zeros((640, L), np.float32)
    OH[seq, np.arange(L)] = 1.0
    OH[128 + cnt, np.arange(L)] = 1.0
    out["emb_oh"] = _blk(OH, 5).astype(np.float16)

    idxw = np.zeros((3, NCHUNK, 128, CH // 16), np.int16)
    for t, k in enumerate(["linear_loc_squares", "up_loc_squares", "down_loc_squares"]):
        IDX = _to_np(ins[k], np.int64)[b]  # (q, k)
        for g in range(8):
            kb, qh = g // 2, g % 2
            sub = IDX[256 * qh:256 * qh + 256, 128 * kb:128 * kb + 128]
            flat = sub.T.reshape(-1)  # j = k_local*256 + q_local
            for c in range(NCHUNK):
                sl = flat[c * CH:(c + 1) * CH]
                idxw[t, c, 16 * g:16 * g + 16, :] = sl.reshape(CH // 16, 16).T
    out["idxw"] = idxw

    # combined causal + key-padding additive mask, (k_local, kb, q) layout
    kg = np.arange(L)
    masked = (kg[:, None] > kg[None, :]) | (seq == PADTOK)[:, None]  # (k, q)
    mck = np.where(masked, NEG, 0.0).astype(np.float16)
    out["maskck"] = np.ascontiguousarray(mck.reshape(4, 128, L).transpose(1, 0, 2))

    ring = (seq == RING_START)
    cs = np.cumsum(ring.astype(np.int64))
    S = np.zeros((L, NRE), np.float32)
    for l in range(L):
        if ring[l] and cs[l] <= NRE:
            S[l, cs[l] - 1] = 1.0
    out["s16d"] = _blk(S, 4).astype(np.float16)

    out["masku"] = np.ascontiguousarray((gm | vm).T).astype(np.uint8)
    return out


last_exec_s = None


def kernel(**inputs):
    global last_exec_s
    if "nc" not in _cached:
        _cached["nc"] = build_program(debug=False)
    nc = _cached["nc"]

    shared = _prep_shared(inputs)
    in_maps = []
    for b in range(B):
        m = dict(shared)
        m.update(_prep_core(inputs, b))
        in_maps.append(m)

    if os.environ.get("KERNEL_SIM"):
        from concourse.bass_interp import CoreSim
        outs = []
        for b in range(B):
            sim = CoreSim(nc, require_finite=False, require_nnan=False)
            for k, v in in_maps[b].items():
                sim.tensor(k)[:] = v
            sim.simulate()
            outs.append(np.array(sim.tensor("logT")).T)
        return np.stack(outs).astype(np.float32)

    from concourse.bass_utils import run_bass_kernel_spmd
    t0 = time.time()
    res = run_bass_kernel_spmd(nc, in_maps, list(range(B)))
    last_exec_s = time.time() - t0
    return np.stack([res.results[b]["logT"].T for b in range(B)]).astype(np.float32)


def bench(inputs, iters=10):
    """Time device execution with inputs resident on device (excludes the
    host->device transfer that dominates kernel() wall time under axon)."""
    import jax
    import jax.numpy as jnp
    from jax.sharding import Mesh, PartitionSpec, NamedSharding
    from jax.experimental.shard_map import shard_map
    import concourse.mybir as mybir_
    from concourse.bass2jax import (
        _bass_exec_p, install_neuronx_cc_hook, partition_id_tensor)

    if "nc" not in _cached:
        _cached["nc"] = build_program(debug=False)
    nc = _cached["nc"]
    install_neuronx_cc_hook()

    shared = _prep_shared(inputs)
    in_maps = []
    for b in range(B):
        m = dict(shared)
        m.update(_prep_core(inputs, b))
        in_maps.append(m)

    pname = nc.partition_id_tensor.name if nc.partition_id_tensor else None
    in_names, out_names, out_avals, zero_outs = [], [], [], []
    for alloc in nc.m.functions[0].allocations:
        if not isinstance(alloc, mybir_.MemoryLocationSet):
            continue
        name = alloc.memorylocations[0].name
        if alloc.kind == "ExternalInput":
            if name != pname:
                in_names.append(name)
        elif alloc.kind == "ExternalOutput":
            out_names.append(name)
            shape = tuple(alloc.tensor_shape)
            dtype = mybir_.dt.np(alloc.dtype)
            out_avals.append(jax.core.ShapedArray(shape, dtype))
            zero_outs.append(np.zeros(shape, dtype))
    n_params = len(in_names)
    all_names = in_names + out_names
    if pname is not None:
        all_names = all_names + [pname]

    def _body(*args):
        operands = list(args)
        if pname is not None:
            operands.append(partition_id_tensor())
        outs = _bass_exec_p.bind(
            *operands,
            out_avals=tuple(out_avals),
            in_names=tuple(all_names),
            out_names=tuple(out_names),
            lowering_input_output_aliases=(),
            sim_require_finite=True,
            sim_require_nnan=True,
            nc=nc,
        )
        return tuple(outs)

    devices = jax.devices()[:B]
    mesh = Mesh(np.asarray(devices), ("core",))
    spec = PartitionSpec("core")
    nio = n_params + len(out_names)
    fn = jax.jit(
        shard_map(_body, mesh=mesh, in_specs=(spec,) * nio,
                  out_specs=(spec,) * len(out_names), check_rep=False),
        keep_unused=True,
    )
    sh = NamedSharding(mesh, spec)
    dev_in = [
        jax.device_put(
            np.concatenate([np.asarray(in_maps[c][nm]) for c in range(B)], 0), sh)
        for nm in in_names
    ]
    dev_zero = [
        jax.device_put(np.zeros((B * z.shape[0], *z.shape[1:]), z.dtype), sh)
        for z in zero_outs
    ]
    # warmup (compile)
    out = fn(*dev_in, *dev_zero)
    jax.block_until_ready(out)
    times = []
    for _ in range(iters):
        t0 = time.time()
        out = fn(*dev_in, *dev_zero)
        jax.block_until_ready(out)
        times.append(time.time() - t0)
    got = np.asarray(out[out_names.index("logT")]).reshape(B, V, L).transpose(0, 2, 1)
    return min(times), sorted(times)[len(times) // 2], got.astype(np.float32)


# revision 38
# speedup vs baseline: 7561.1512x; 114.5111x over previous
"""Trainium2 Bass kernel for nn_BaseGenerator_38989713113442.

6-layer post-norm transformer encoder (B=8, L=512, E=512, H=8, FFN=2048) with a
gathered per-head attention bias (three 513-entry table lookups over (B,L,L)
index tensors) and an edge-logit head (cumsum scatter + bilinear logits).

Strategy: data-parallel over batch B across the 8 NeuronCores (one row per
core).  Activations are kept transposed on-chip (feature dim on partitions,
L=512 on the free dim) so every matmul consumes the previous output without
transposes.  Matmuls run in fp16 with fp32 PSUM accumulation; layernorm /
softmax bookkeeping stays fp32.  The attention-bias table gathers run on the
GpSimd engine (ap_gather), are summed on DVE, round-trip through a DRAM bounce
buffer to land in (k-partition, q-free) layout, and get causal/padding masks
applied in place.  Host-side work is layout/index preprocessing only (one-hot
encodings of integer inputs, wrapped int16 gather indices, transposed weights,
mask tensors); all FLOPs happen on-device.
"""

import math
import os
import sys
import time

sys.path.insert(0, "/opt/trn_rl_repo")

import numpy as np

import concourse.bacc as bacc
import concourse.mybir as mybir
from concourse.tile import TileContext

B, L = 8, 512
E, H, HD = 512, 8, 64
FF, NL = 2048, 6
V, NRE, RING_START, PADTOK, MAXLEN = 45, 20, 24, 0, 512
NOUT0 = V - NRE  # 25
EBN, FBN = 4, 16  # 128-blocks in E and FF
NEG = float("-inf")
SCALE_E = math.sqrt(float(E))
INV_SQRT_HD = 1.0 / math.sqrt(float(HD))  # folded into k-projection weights
INV_SQRT_E = E ** -0.5

F16 = mybir.dt.float16
F32 = mybir.dt.float32
I16 = mybir.dt.int16
U8 = mybir.dt.uint8
AF = mybir.ActivationFunctionType
ALU = mybir.AluOpType

CH = 4096                      # gather idxs per group per call
NCHUNK = (128 * 256) // CH     # 8 chunks cover 128 k x 256 q per group
# gather groups (kb, qh); the two fully causal-masked ones are skipped
G_ORDER = [(0, 0), (0, 1), (1, 0), (1, 1), (2, 1), (3, 1)]
MASKED_GROUPS = [(2, 0), (3, 0)]

_cached = {}


def build_program(debug=False):
    _e = os.environ.get
    PSMM = int(_e("K_PSMM", "4"))
    PSAUX = int(_e("K_PSAUX", "2"))
    PSROW = int(_e("K_PSROW", "1"))
    ATB = int(_e("K_ATB", "4"))
    nc = bacc.Bacc("TRN2", target_bir_lowering=False, debug=False)
    dt = nc.dram_tensor

    emb_tab = dt("emb_tab", [128, 5, E], F16, kind="ExternalInput")
    emb_oh = dt("emb_oh", [128, 5, L], F16, kind="ExternalInput")
    wqkv = dt("wqkv", [NL, 128, 4, 1024], F16, kind="ExternalInput")
    bqk = dt("bqk", [128, NL, 8], F32, kind="ExternalInput")
    wv = dt("wv", [NL, 128, 5, E], F16, kind="ExternalInput")
    wout = dt("wout", [NL, 128, 4, E], F16, kind="ExternalInput")
    bout = dt("bout", [128, NL, 4], F32, kind="ExternalInput")
    wf1 = dt("wf1", [NL, 128, 4, FF], F16, kind="ExternalInput")
    bf1 = dt("bf1", [128, NL, 16], F32, kind="ExternalInput")
    wf2 = dt("wf2", [NL, 128, 16, E], F16, kind="ExternalInput")
    bf2 = dt("bf2", [128, NL, 4], F32, kind="ExternalInput")
    lnp = dt("lnp", [128, 13, 2, 4], F32, kind="ExternalInput")
    tabs = dt("tabs", [128, 3, 513], F32, kind="ExternalInput")
    idxw = dt("idxw", [3, NCHUNK, 96, CH // 16], I16, kind="ExternalInput")
    maskck = dt("maskck", [128, 4, L], F16, kind="ExternalInput")
    wgen = dt("wgen", [128, 4, NOUT0], F16, kind="ExternalInput")
    bgen = dt("bgen", [NOUT0, 1], F32, kind="ExternalInput")
    w0r = dt("w0r", [128, 4, E], F16, kind="ExternalInput")
    b0c = dt("b0c", [128, 4], F32, kind="ExternalInput")
    w1r = dt("w1r", [128, 5, E], F16, kind="ExternalInput")
    s16d = dt("s16d", [128, 4, NRE], F16, kind="ExternalInput")
    masku = dt("masku", [V, L], U8, kind="ExternalInput")
    logT = dt("logT", [V, L], F32, kind="ExternalOutput")
    dbg = dt("dbg", [NL + 1, 128, EBN, L], F16, kind="ExternalOutput") if debug else None

    with TileContext(nc) as tc:
        with tc.tile_pool(name="persist", bufs=1) as pp, \
             tc.tile_pool(name="rows", bufs=1) as rowp, \
             tc.tile_pool(name="psmm", bufs=PSMM, space="PSUM") as psmm, \
             tc.tile_pool(name="psaux", bufs=PSAUX, space="PSUM") as psaux, \
             tc.tile_pool(name="psrow", bufs=PSROW, space="PSUM") as psrow:

            biasT = pp.tile([128, H, 4, L], F16)        # [k_local, h, kb, q]
            x16 = pp.tile([128, EBN, L], F16)           # x transposed, fp16
            ones16 = pp.tile([128, L], F16)             # row 0 = 1.0, else 0
            onesc16 = pp.tile([128, 1], F16)            # all 1.0 (stats lhsT)
            onesr16 = rowp.tile([1, 128], F16)          # all 1.0 (bcast lhsT)
            epsr = rowp.tile([1, 1], F32)               # layernorm epsilon

            nc.vector.memset(epsr[:], 1e-5)
            nc.vector.memset(ones16[:], 0.0)
            nc.vector.memset(ones16[0:1, :], 1.0)
            nc.vector.memset(onesc16[:], 1.0)
            nc.vector.memset(onesr16[:], 1.0)

            # ---------------- Phase G: attention bias gather -------------
            with tc.tile_pool(name="gat", bufs=1) as gp, \
                 tc.tile_pool(name="gidx", bufs=6) as gip, \
                 tc.tile_pool(name="gout", bufs=2) as gop, \
                 tc.tile_pool(name="gacc", bufs=3) as gap, \
                 tc.tile_pool(name="gdram", bufs=1, space="DRAM") as gdp:
                tabs_s = gp.tile([128, 3, 513], F32)
                nc.sync.dma_start(tabs_s[:], tabs[:])
                NGRP = len(G_ORDER)  # 6 useful groups (2 fully causal-masked skipped)
                NCHAN = 16 * NGRP
                bounce = gdp.tile([NCHUNK, NCHAN, CH], F32)
                for c in range(NCHUNK):
                    gacc = gap.tile([NCHAN, CH], F32, tag="gacc")
                    for t in range(3):
                        ix = gip.tile([NCHAN, CH // 16], I16, tag="gidx")
                        nc.sync.dma_start(ix[:], idxw[t, c])
                        if t == 0:
                            nc.gpsimd.ap_gather(
                                gacc[:], tabs_s[0:NCHAN, t, :], ix[:],
                                channels=NCHAN, num_elems=513, d=1, num_idxs=CH)
                        else:
                            gt = gop.tile([NCHAN, CH], F32, tag="gt")
                            nc.gpsimd.ap_gather(
                                gt[:], tabs_s[0:NCHAN, t, :], ix[:],
                                channels=NCHAN, num_elems=513, d=1, num_idxs=CH)
                            nc.vector.tensor_tensor(gacc[:], gacc[:], gt[:], op=ALU.add)
                    nc.sync.dma_start(bounce[c], gacc[:])
                # fully-masked (kb, qh) pieces: zero them (mask add writes -inf)
                for kb, qh in MASKED_GROUPS:
                    nc.vector.memset(biasT[:, :, kb, 256 * qh:256 * qh + 256], 0.0)
                # redistribute: bounce[c][16g+h, kk*256+q] -> biasT[16c+kk, h, kb, qh*256+q]
                for c in range(NCHUNK):
                    for g, (kb, qh) in enumerate(G_ORDER):
                        srcv = bounce[c][16 * g:16 * g + 8].rearrange(
                            "h (kk q) -> kk h q", kk=16)
                        dstv = biasT[16 * c:16 * c + 16, :, kb,
                                     256 * qh:256 * qh + 256]
                        nc.gpsimd.dma_start(dstv, srcv)
                # combined causal + key-padding additive mask (-inf / 0)
                mck_s = gp.tile([128, 4, L], F16)
                nc.sync.dma_start(mck_s[:], maskck[:])
                for h in range(H):
                    for kb in range(4):
                        nc.vector.tensor_tensor(
                            biasT[:, h, kb, :], biasT[:, h, kb, :],
                            mck_s[:, kb, :], op=ALU.add)
                # biasT := exp(bias + mask); the per-layer softmax then uses
                # exp(s + b) = exp(s) * exp(b) with a fast all-fp16 multiply
                for h in range(H):
                    nc.scalar.activation(biasT[:, h, :, :], biasT[:, h, :, :], AF.Exp)

            # ---------------- Phase E: embeddings ------------------------
            with tc.tile_pool(name="emb", bufs=1) as ep:
                et = ep.tile([128, 5, E], F16)
                eo = ep.tile([128, 5, L], F16)
                nc.sync.dma_start(et[:], emb_tab[:])
                nc.sync.dma_start(eo[:], emb_oh[:])
                for eb in range(EBN):
                    ps = psmm.tile([128, L], F32, tag="mm")
                    for kb in range(5):
                        nc.tensor.matmul(
                            ps[:], et[:, kb, eb * 128:(eb + 1) * 128], eo[:, kb, :],
                            start=(kb == 0), stop=(kb == 4))
                    nc.scalar.activation(x16[:, eb, :], ps[:], AF.Copy, scale=SCALE_E)
            if debug:
                nc.sync.dma_start(dbg[0], x16[:])

            # ---------------- layers -------------------------------------
            with tc.tile_pool(name="wq", bufs=2) as wqp, \
                 tc.tile_pool(name="wv", bufs=2) as wvp, \
                 tc.tile_pool(name="wo", bufs=2) as wop, \
                 tc.tile_pool(name="wf1", bufs=1) as wf1p, \
                 tc.tile_pool(name="wf2", bufs=1) as wf2p, \
                 tc.tile_pool(name="lay", bufs=1) as lp, \
                 tc.tile_pool(name="att", bufs=ATB) as ap_, \
                 tc.tile_pool(name="bias_s", bufs=1) as bsp:

                bias_all = bsp.tile([128, 13, 2, 4], F32)
                nc.sync.dma_start(bias_all[:], lnp[:])
                bqk_s = bsp.tile([128, NL, 8], F32)
                nc.sync.dma_start(bqk_s[:], bqk[:])
                bout_s = bsp.tile([128, NL, 4], F32)
                nc.sync.dma_start(bout_s[:], bout[:])
                bf1_s = bsp.tile([128, NL, 16], F32)
                nc.sync.dma_start(bf1_s[:], bf1[:])
                bf2_s = bsp.tile([128, NL, 4], F32)
                nc.sync.dma_start(bf2_s[:], bf2[:])

                def layernorm(src16, ln_idx):
                    """src16 (128, EBN, L) fp16 pre-LN -> writes x16 post-LN."""
                    sq = lp.tile([128, EBN, L], F16, tag="sq")
                    for eb in range(EBN):
                        nc.vector.tensor_tensor(
                            sq[:, eb, :], src16[:, eb, :], src16[:, eb, :], op=ALU.mult)
                    psm = psrow.tile([1, L], F32, tag="st")
                    for eb in range(EBN):
                        nc.tensor.matmul(psm[:], onesc16[:], src16[:, eb, :],
                                         start=(eb == 0), stop=(eb == 3))
                    psv = psrow.tile([1, L], F32, tag="st")
                    for eb in range(EBN):
                        nc.tensor.matmul(psv[:], onesc16[:], sq[:, eb, :],
                                         start=(eb == 0), stop=(eb == 3))
                    rw = rowp.tile([1, 3, L], F32, tag="lnrows")
                    r16 = rowp.tile([1, 2, L], F16, tag="lnrows16")
                    nc.vector.tensor_scalar_mul(r16[0:1, 0, :], psm[0:1, :], 1.0 / E)
                    nc.vector.tensor_tensor(rw[0:1, 0, :], psm[0:1, :], r16[0:1, 0, :], op=ALU.mult)
                    nc.vector.tensor_tensor(rw[0:1, 1, :], psv[0:1, :], rw[0:1, 0, :], op=ALU.subtract)
                    nc.scalar.activation(rw[0:1, 2, :], rw[0:1, 1, :], AF.Sqrt,
                                         bias=epsr[:], scale=1.0 / E)
                    nc.vector.reciprocal(rw[0:1, 1, :], rw[0:1, 2, :])
                    nc.vector.tensor_copy(r16[0:1, 1, :], rw[0:1, 1, :])
                    psbm = psrow.tile([128, L], F32, tag="bc")
                    nc.tensor.matmul(psbm[:], onesr16[:], r16[0:1, 0, :], start=True, stop=True)
                    psbr = psrow.tile([128, L], F32, tag="bc")
                    nc.tensor.matmul(psbr[:], onesr16[:], r16[0:1, 1, :], start=True, stop=True)
                    for eb in range(EBN):
                        t32 = ap_.tile([128, L], F32, tag="t32")
                        nc.vector.tensor_tensor(
                            t32[:], src16[:, eb, :], psbm[:], op=ALU.subtract)
                        nc.vector.tensor_tensor(t32[:], t32[:], psbr[:], op=ALU.mult)
                        g_col = bias_all[:, ln_idx, 0, eb:eb + 1]
                        b_col = bias_all[:, ln_idx, 1, eb:eb + 1]
                        nc.scalar.activation(
                            x16[:, eb, :], t32[:], AF.Identity, bias=b_col, scale=g_col)

                for li in range(NL):
                    wq_s = wqp.tile([128, 4, 1024], F16, tag="wq")
                    nc.sync.dma_start(wq_s[:], wqkv[li])
                    wv_s = wvp.tile([128, 5, E], F16, tag="wv")
                    nc.sync.dma_start(wv_s[:], wv[li])
                    wo_s = wop.tile([128, 4, E], F16, tag="wo")
                    nc.sync.dma_start(wo_s[:], wout[li])

                    # q/k projections (k pre-scaled by 1/sqrt(hd) on host)
                    qk = lp.tile([128, 8, L], F16, tag="qk")
                    for m in range(8):
                        ps = psmm.tile([128, L], F32, tag="mm")
                        for kb in range(4):
                            nc.tensor.matmul(
                                ps[:], wq_s[:, kb, m * 128:(m + 1) * 128], x16[:, kb, :],
                                start=(kb == 0), stop=(kb == 3))
                        nc.vector.tensor_scalar_add(qk[:, m, :], ps[:],
                                                    bqk_s[:, li, m:m + 1])
                    # v (untransposed: l on partitions) + ones column for sums
                    v16 = lp.tile([128, 4, H, HD + 1], F16, tag="v16")
                    nc.vector.memset(v16[:, :, :, HD:HD + 1], 1.0)
                    for lb in range(4):
                        ps = psmm.tile([128, L], F32, tag="mm")
                        for kb in range(5):
                            lhs = (x16[:, kb, lb * 128:(lb + 1) * 128] if kb < 4
                                   else ones16[:, lb * 128:(lb + 1) * 128])
                            nc.tensor.matmul(ps[:], lhs, wv_s[:, kb, :],
                                             start=(kb == 0), stop=(kb == 4))
                        nc.scalar.activation(
                            v16[:, lb, :, 0:HD],
                            ps[:].rearrange("p (h d) -> p h d", d=HD), AF.Copy)

                    ctx16 = lp.tile([128, EBN, L], F16, tag="ctx")
                    for h in range(H):
                        po = (h % 2) * 64
                        mq, mk = h // 2, 4 + h // 2
                        aT = ap_.tile([128, 4, L], F16, tag="aT")
                        for kb in range(4):
                            # causal: keys in block kb only attend queries q >= 128*kb
                            q0 = 128 * kb
                            psA = psmm.tile([128, L], F32, tag="mm")
                            nc.tensor.matmul(
                                psA[:, 0:L - q0],
                                qk[po:po + 64, mk, kb * 128:(kb + 1) * 128],
                                qk[po:po + 64, mq, q0:L],
                                start=True, stop=True)
                            eS = ap_.tile([128, L], F16, tag="eS")
                            nc.scalar.activation(eS[:, 0:L - q0], psA[:, 0:L - q0],
                                                 AF.Exp)
                            nc.vector.tensor_tensor(
                                aT[:, kb, q0:L], eS[:, 0:L - q0],
                                biasT[:, h, kb, q0:L], op=ALU.mult)
                        psC = psaux.tile([HD + 1, L], F32, tag="ctxp")
                        for qr in range(4):
                            for kb in range(qr + 1):
                                nc.tensor.matmul(
                                    psC[:, 128 * qr:128 * (qr + 1)],
                                    v16[:, kb, h, :],
                                    aT[:, kb, 128 * qr:128 * (qr + 1)],
                                    start=(kb == 0), stop=(kb == qr))
                        # reciprocal of sums (row 64) at matching partitions, then
                        # DMA the fp16 row down to partition 0 for the broadcast
                        rc64 = ap_.tile([HD + 1, L], F32, tag="rc64")
                        r1664 = ap_.tile([HD + 1, L], F16, tag="r1664")
                        rrow = ap_.tile([1, L], F16, tag="rrow")
                        nc.vector.reciprocal(rc64[HD:HD + 1, :], psC[HD:HD + 1, :])
                        nc.vector.tensor_copy(r1664[HD:HD + 1, :], rc64[HD:HD + 1, :])
                        nc.sync.dma_start(rrow[:], r1664[HD:HD + 1, :])
                        psR = psrow.tile([128, L], F32, tag="bc")
                        nc.tensor.matmul(psR[:], onesr16[:], rrow[:],
                                         start=True, stop=True)
                        rb16 = ap_.tile([128, L], F16, tag="rb16")
                        nc.scalar.activation(rb16[:], psR[:], AF.Copy)
                        ctxh = ap_.tile([HD, L], F16, tag="ctxh")
                        nc.vector.tensor_tensor(
                            ctxh[:], psC[0:HD, :], rb16[0:HD, :], op=ALU.mult)
                        nc.sync.dma_start(ctx16[po:po + 64, h // 2, :], ctxh[:])

                    res16 = lp.tile([128, EBN, L], F16, tag="res")
                    for eb in range(EBN):
                        ps = psmm.tile([128, L], F32, tag="mm")
                        for kb in range(4):
                            nc.tensor.matmul(
                                ps[:], wo_s[:, kb, eb * 128:(eb + 1) * 128],
                                ctx16[:, kb, :], start=(kb == 0), stop=(kb == 3))
                        pb = ap_.tile([128, L], F16, tag="pb16")
                        nc.scalar.activation(pb[:], ps[:], AF.Identity,
                                             bias=bout_s[:, li, eb:eb + 1])
                        nc.vector.tensor_tensor(
                            res16[:, eb, :], pb[:], x16[:, eb, :], op=ALU.add)
                    layernorm(res16, 2 * li)

                    # FFN
                    w1_s = wf1p.tile([128, 4, FF], F16, tag="wf1")
                    nc.sync.dma_start(w1_s[:], wf1[li])
                    hT = lp.tile([128, FBN, L], F16, tag="hT")
                    for fb in range(FBN):
                        ps = psmm.tile([128, L], F32, tag="mm")
                        for kb in range(4):
                            nc.tensor.matmul(
                                ps[:], w1_s[:, kb, fb * 128:(fb + 1) * 128],
                                x16[:, kb, :], start=(kb == 0), stop=(kb == 3))
                        nc.scalar.activation(hT[:, fb, :], ps[:], AF.Gelu,
                                             bias=bf1_s[:, li, fb:fb + 1])
                    w2_s = wf2p.tile([128, 16, E], F16, tag="wf2")
                    nc.sync.dma_start(w2_s[:], wf2[li])
                    for eb in range(EBN):
                        ps = psmm.tile([128, L], F32, tag="mm")
                        for kb in range(16):
                            nc.tensor.matmul(
                                ps[:], w2_s[:, kb, eb * 128:(eb + 1) * 128],
                                hT[:, kb, :], start=(kb == 0), stop=(kb == 15))
                        pb = ap_.tile([128, L], F16, tag="pb16")
                        nc.scalar.activation(pb[:], ps[:], AF.Identity,
                                             bias=bf2_s[:, li, eb:eb + 1])
                        nc.vector.tensor_tensor(
                            res16[:, eb, :], pb[:], x16[:, eb, :], op=ALU.add)
                    layernorm(res16, 2 * li + 1)
                    if debug:
                        nc.sync.dma_start(dbg[li + 1], x16[:])

                # final LN (x16 copied so the apply can overwrite x16)
                fin16 = lp.tile([128, EBN, L], F16, tag="res")
                for eb in range(EBN):
                    nc.vector.tensor_copy(fin16[:, eb, :], x16[:, eb, :])
                layernorm(fin16, 12)

                # ------------- head (reuses layer pool slots) -----------
                if True:
                    logsb0 = lp.tile([NOUT0, L], F32, tag="logsb0")
                    logsb1 = lp.tile([NRE, L], F32, tag="logsb1")
                    wg_s = lp.tile([128, 4, NOUT0], F16, tag="wgen")
                    nc.sync.dma_start(wg_s[:], wgen[:])
                    bg_s = lp.tile([NOUT0, 1], F32, tag="bgen")
                    nc.sync.dma_start(bg_s[:], bgen[:])
                    psG = psaux.tile([NOUT0, L], F32, tag="ctxp")
                    for kb in range(4):
                        nc.tensor.matmul(psG[:], wg_s[:, kb, :], x16[:, kb, :],
                                         start=(kb == 0), stop=(kb == 3))
                    nc.scalar.activation(logsb0[:], psG[:], AF.Identity, bias=bg_s[:])

                    w0_s = wop.tile([128, 4, E], F16, tag="wo")
                    nc.sync.dma_start(w0_s[:], w0r[:])
                    b0_s = lp.tile([128, 4], F32, tag="b0c")
                    nc.sync.dma_start(b0_s[:], b0c[:])
                    o0 = lp.tile([128, EBN, L], F16, tag="ctx")
                    for eb in range(EBN):
                        ps = psmm.tile([128, L], F32, tag="mm")
                        for kb in range(4):
                            nc.tensor.matmul(
                                ps[:], w0_s[:, kb, eb * 128:(eb + 1) * 128],
                                x16[:, kb, :], start=(kb == 0), stop=(kb == 3))
                        nc.scalar.activation(o0[:, eb, :], ps[:], AF.Identity,
                                             bias=b0_s[:, eb:eb + 1])
                    w1_s = wvp.tile([128, 5, E], F16, tag="wv")
                    nc.sync.dma_start(w1_s[:], w1r[:])
                    o1 = lp.tile([128, 4, E], F16, tag="sq")
                    for lb in range(4):
                        ps = psmm.tile([128, L], F32, tag="mm")
                        for kb in range(5):
                            lhs = (x16[:, kb, lb * 128:(lb + 1) * 128] if kb < 4
                                   else ones16[:, lb * 128:(lb + 1) * 128])
                            nc.tensor.matmul(ps[:], lhs, w1_s[:, kb, :],
                                             start=(kb == 0), stop=(kb == 4))
                        nc.vector.tensor_copy(o1[:, lb, :], ps[:])
                    s_s = lp.tile([128, 4, NRE], F16, tag="s16")
                    nc.sync.dma_start(s_s[:], s16d[:])
                    rt = lp.tile([128, EBN, NRE], F16, tag="rt")
                    for eb in range(EBN):
                        ps = psaux.tile([128, NRE], F32, tag="ctxp")
                        for lb in range(4):
                            nc.tensor.matmul(
                                ps[:], o1[:, lb, eb * 128:(eb + 1) * 128],
                                s_s[:, lb, :], start=(lb == 0), stop=(lb == 3))
                        nc.vector.tensor_copy(rt[:, eb, :], ps[:])
                    psL = psaux.tile([NRE, L], F32, tag="ctxp")
                    for eb in range(EBN):
                        nc.tensor.matmul(psL[:], rt[:, eb, :], o0[:, eb, :],
                                         start=(eb == 0), stop=(eb == 3))
                    nc.scalar.activation(logsb1[:], psL[:], AF.Copy,
                                         scale=INV_SQRT_E)
                    # output masking: where(mask, -inf, logits)
                    mk0 = lp.tile([NOUT0, L], U8, tag="mk0")
                    mk1 = lp.tile([NRE, L], U8, tag="mk1")
                    nc.sync.dma_start(mk0[:], masku[0:NOUT0])
                    nc.sync.dma_start(mk1[:], masku[NOUT0:V])
                    ninf = lp.tile([NOUT0, L], F32, tag="ninf")
                    nc.vector.memset(ninf[:], NEG)
                    nc.vector.copy_predicated(logsb0[:], mk0[:], ninf[:])
                    nc.vector.copy_predicated(logsb1[:], mk1[:], ninf[0:NRE, :])
                    nc.sync.dma_start(logT[0:NOUT0, :], logsb0[:])
                    nc.sync.dma_start(logT[NOUT0:V, :], logsb1[:])

    nc.compile()
    return nc


def _to_np(x, dtype=None):
    a = np.asarray(x)
    if dtype is not None:
        a = a.astype(dtype)
    return a


def _blk(a, nb):
    """(nb*128, X) -> (128, nb, X)"""
    return np.ascontiguousarray(a.reshape(nb, 128, -1).transpose(1, 0, 2))


def _prep_shared(ins):
    f = lambda k: _to_np(ins[k], np.float32)
    out = {}
    # embedding table: rows 0..44 tok, 128..639 cnt
    TAB = np.zeros((640, E), np.float32)
    TAB[0:V] = f("tok_emb")
    TAB[128:128 + MAXLEN] = f("cnt_emb")
    out["emb_tab"] = _blk(TAB, 5).astype(np.float16)

    ipw, ipb = f("in_proj_w"), f("in_proj_b")
    wqkv = np.zeros((NL, 128, 4, 1024), np.float16)
    bqk = np.zeros((128, NL, 8), np.float32)
    wv = np.zeros((NL, 128, 5, E), np.float16)
    wout_ = np.zeros((NL, 128, 4, E), np.float16)
    bout = np.zeros((128, NL, 4), np.float32)
    wf1 = np.zeros((NL, 128, 4, FF), np.float16)
    bf1 = np.zeros((128, NL, 16), np.float32)
    wf2 = np.zeros((NL, 128, 16, E), np.float16)
    bf2 = np.zeros((128, NL, 4), np.float32)
    lnp = np.zeros((128, 13, 2, 4), np.float32)
    for i in range(NL):
        wq = ipw[i, 0:E]                     # (512, 512)
        wk = ipw[i, E:2 * E] * INV_SQRT_HD
        cat = np.concatenate([wq, wk], 0).T  # (512, 1024) = lhsT
        wqkv[i] = _blk(cat, 4).astype(np.float16)
        bcat = np.concatenate([ipb[i, 0:E], ipb[i, E:2 * E] * INV_SQRT_HD])
        bqk[:, i, :] = bcat.reshape(8, 128).T
        wvt = np.zeros((640, E), np.float32)
        wvt[0:E] = ipw[i, 2 * E:3 * E].T
        wvt[E] = ipb[i, 2 * E:3 * E]
        wv[i] = _blk(wvt, 5).astype(np.float16)
        wout_[i] = _blk(f("out_w")[i].T, 4).astype(np.float16)
        bout[:, i, :] = f("out_b")[i].reshape(4, 128).T
        wf1[i] = _blk(f("ffn_w1")[i].T, 4).astype(np.float16)
        bf1[:, i, :] = f("ffn_b1")[i].reshape(16, 128).T
        wf2[i] = _blk(f("ffn_w2")[i].T, 16).astype(np.float16)
        bf2[:, i, :] = f("ffn_b2")[i].reshape(4, 128).T
        lnp[:, 2 * i, 0, :] = f("ln1_g")[i].reshape(4, 128).T
        lnp[:, 2 * i, 1, :] = f("ln1_b")[i].reshape(4, 128).T
        lnp[:, 2 * i + 1, 0, :] = f("ln2_g")[i].reshape(4, 128).T
        lnp[:, 2 * i + 1, 1, :] = f("ln2_b")[i].reshape(4, 128).T
    lnp[:, 12, 0, :] = f("fin_g").reshape(4, 128).T
    lnp[:, 12, 1, :] = f("fin_b").reshape(4, 128).T
    out.update(wqkv=wqkv, bqk=bqk, wv=wv, wout=wout_, bout=bout,
               wf1=wf1, bf1=bf1, wf2=wf2, bf2=bf2, lnp=lnp)

    tabs = np.zeros((128, 3, 513), np.float32)
    for t, k in enumerate(["lin_loc_emb", "up_loc_emb", "down_loc_emb"]):
        tb = f(k)  # (513, 8)
        for p in range(128):
            tabs[p, t, :] = tb[:, (p % 16) % 8]
    out["tabs"] = tabs

    out["wgen"] = _blk(f("gen_w").T, 4).astype(np.float16)
    out["bgen"] = f("gen_b").reshape(NOUT0, 1)
    out["w0r"] = _blk(f("ring_w0").T, 4).astype(np.float16)
    out["b0c"] = f("ring_b0").reshape(4, 128).T.copy()
    w1t = np.zeros((640, E), np.float32)
    w1t[0:E] = f("ring_w1").T
    w1t[E] = f("ring_b1")
    out["w1r"] = _blk(w1t, 5).astype(np.float16)
    return out


def _prep_core(ins, b):
    seq = _to_np(ins["sequences"], np.int64)[b]
    cnt = _to_np(ins["count_sequences"], np.int64)[b]
    gm = _to_np(ins["graph_mask_sequences"], bool)[b]
    vm = _to_np(ins["valence_mask_sequences"], bool)[b]
    out = {}
    OH = np.zeros((640, L), np.float32)
    OH[seq, np.arange(L)] = 1.0
    OH[128 + cnt, np.arange(L)] = 1.0
    out["emb_oh"] = _blk(OH, 5).astype(np.float16)

    idxw = np.zeros((3, NCHUNK, 96, CH // 16), np.int16)
    for t, k in enumerate(["linear_loc_squares", "up_loc_squares", "down_loc_squares"]):
        IDX = _to_np(ins[k], np.int64)[b]  # (q, k)
        for g, (kb, qh) in enumerate(G_ORDER):
            sub = IDX[256 * qh:256 * qh + 256, 128 * kb:128 * kb + 128]
            flat = sub.T.reshape(-1)  # j = k_local*256 + q_local
            for c in range(NCHUNK):
                sl = flat[c * CH:(c + 1) * CH]
                idxw[t, c, 16 * g:16 * g + 16, :] = sl.reshape(CH // 16, 16).T
    out["idxw"] = idxw

    # combined causal + key-padding additive mask, (k_local, kb, q) layout
    kg = np.arange(L)
    masked = (kg[:, None] > kg[None, :]) | (seq == PADTOK)[:, None]  # (k, q)
    mck = np.where(masked, NEG, 0.0).astype(np.float16)
    out["maskck"] = np.ascontiguousarray(mck.reshape(4, 128, L).transpose(1, 0, 2))

    ring = (seq == RING_START)
    cs = np.cumsum(ring.astype(np.int64))
    S = np.zeros((L, NRE), np.float32)
    for l in range(L):
        if ring[l] and cs[l] <= NRE:
            S[l, cs[l] - 1] = 1.0
    out["s16d"] = _blk(S, 4).astype(np.float16)

    out["masku"] = np.ascontiguousarray((gm | vm).T).astype(np.uint8)
    return out


last_exec_s = None


def kernel(**inputs):
    global last_exec_s
    if "nc" not in _cached:
        _cached["nc"] = build_program(debug=False)
    nc = _cached["nc"]

    shared = _prep_shared(inputs)
    in_maps = []
    for b in range(B):
        m = dict(shared)
        m.update(_prep_core(inputs, b))
        in_maps.append(m)

    if os.environ.get("KERNEL_SIM"):
        from concourse.bass_interp import CoreSim
        outs = []
        for b in range(B):
            sim = CoreSim(nc, require_finite=False, require_nnan=False)
            for k, v in in_maps[b].items():
                sim.tensor(k)[:] = v
            sim.simulate()
            outs.append(np.array(sim.tensor("logT")).T)
        return np.stack(outs).astype(np.float32)

    from concourse.bass_utils import run_bass_kernel_spmd
    t0 = time.time()
    res = run_bass_kernel_spmd(nc, in_maps, list(range(B)))
    last_exec_s = time.time() - t0
    return np.stack([res.results[b]["logT"].T for b in range(B)]).astype(np.float32)


def bench(inputs, iters=10):
    """Time device execution with inputs resident on device (excludes the
    host->device transfer that dominates kernel() wall time under axon)."""
    import jax
    import jax.numpy as jnp
    from jax.sharding import Mesh, PartitionSpec, NamedSharding
    from jax.experimental.shard_map import shard_map
    import concourse.mybir as mybir_
    from concourse.bass2jax import (
        _bass_exec_p, install_neuronx_cc_hook, partition_id_tensor)

    if "nc" not in _cached:
        _cached["nc"] = build_program(debug=False)
    nc = _cached["nc"]
    install_neuronx_cc_hook()

    shared = _prep_shared(inputs)
    in_maps = []
    for b in range(B):
        m = dict(shared)
        m.update(_prep_core(inputs, b))
        in_maps.append(m)

    pname = nc.partition_id_tensor.name if nc.partition_id_tensor else None
    in_names, out_names, out_avals, zero_outs = [], [], [], []
    for alloc in nc.m.functions[0].allocations:
        if not isinstance(alloc, mybir_.MemoryLocationSet):
            continue
        name = alloc.memorylocations[0].name
        if alloc.kind == "ExternalInput":
            if name != pname:
                in_names.append(name)
        elif alloc.kind == "ExternalOutput":
            out_names.append(name)
            shape = tuple(alloc.tensor_shape)
            dtype = mybir_.dt.np(alloc.dtype)
            out_avals.append(jax.core.ShapedArray(shape, dtype))
            zero_outs.append(np.zeros(shape, dtype))
    n_params = len(in_names)
    all_names = in_names + out_names
    if pname is not None:
        all_names = all_names + [pname]

    def _body(*args):
        operands = list(args)
        if pname is not None:
            operands.append(partition_id_tensor())
        outs = _bass_exec_p.bind(
            *operands,
            out_avals=tuple(out_avals),
            in_names=tuple(all_names),
            out_names=tuple(out_names),
            lowering_input_output_aliases=(),
            sim_require_finite=True,
            sim_require_nnan=True,
            nc=nc,
        )
        return tuple(outs)

    devices = jax.devices()[:B]
    mesh = Mesh(np.asarray(devices), ("core",))
    spec = PartitionSpec("core")
    nio = n_params + len(out_names)
    fn = jax.jit(
        shard_map(_body, mesh=mesh, in_specs=(spec,) * nio,
                  out_specs=(spec,) * len(out_names), check_rep=False),
        keep_unused=True,
    )
    sh = NamedSharding(mesh, spec)
    dev_in = [
        jax.device_put(
            np.concatenate([np.asarray(in_maps[c][nm]) for c in range(B)], 0), sh)
        for nm in in_names
    ]
    dev_zero = [
        jax.device_put(np.zeros((B * z.shape[0], *z.shape[1:]), z.dtype), sh)
        for z in zero_outs
    ]
    # warmup (compile)
    out = fn(*dev_in, *dev_zero)
    jax.block_until_ready(out)
    times = []
    for _ in range(iters):
        t0 = time.time()
        out = fn(*dev_in, *dev_zero)
        jax.block_until_ready(out)
        times.append(time.time() - t0)
    got = np.asarray(out[out_names.index("logT")]).reshape(B, V, L).transpose(0, 2, 1)
    return min(times), sorted(times)[len(times) // 2], got.astype(np.float32)
